# revision 1
# baseline (speedup 1.0000x reference)
"""Binarized LeNet5+BN forward on 8 Trainium2 NeuronCores.

Strategy (data-parallel over batch, 1024 images/core):
  * Everything on-chip is feature-major: [features on partitions, batch on free dim].
  * sign(x) / sign(hardtanh(bn(z))) == sign(scale*z + bias): each layer is
    matmul-accumulate into PSUM followed by ONE ScalarE activation
    (Sign with per-partition scale/bias = fused conv-bias+BN+hardtanh+sign).
  * The input is signed + laid out feature-major on the HOST (pointwise prep,
    like the host-built Toeplitz weight matrices): 4 phase-shifted fp8 copies
    [4,128,7,B] so every conv1 operand sits at partition base 0.
  * Convs are row-Toeplitz matmuls: weights are +-1/0 matrices built host-side;
    activations are y-major with image rows padded to 32 partitions. Conv output
    row y contracts rows [32y, 32y+160) of the previous layer.
  * All conv/fc1 matmul operands are fp8e4 (+-1/0 exact; PSUM accumulates fp32)
    with perf_mode=DoubleRow: each matmul contracts TWO K<=128 tiles (slices of
    the activation tensor paired along the free dim), halving PE passes.
    Measured DR cadence equals a plain N=512 matmul => 2x throughput.
  * conv1's 16-row remainder outputs land in one PSUM bank at 16-feature
    column offsets (odd offsets via a zero-padded 32-wide weight + accumulate),
    so ONE activation per 8 rows writes the 16-stride remainder bundle
    directly. Bundles live in the same tensor as the main activations (slots
    24+3p+s, 8 phase-shifted copies via SBUF DMA), so conv2 needs only THREE
    DoubleRow matmuls per (y2, Mt): (ky0,ky1), (ky2,ky3), (ky4, remainder).
    Padding lanes produce sign(garbage)=+-1 nulled by zero weight rows.
  * All weights/scales are packed into 3 DMA loads issued AFTER the first
    input tile loads, so the PE starts ~8us into the kernel.
  * Double-buffered pools let chunk c+1's input DMAs run under chunk c's
    matmuls, keeping the PE dense and the HAM clock un-throttled.
  * fc1 weights are column-permuted host-side to match the on-chip act2 layout;
    the final output stays feature-major [10, B] and is transposed on host.
"""

from contextlib import ExitStack

import ml_dtypes
import numpy as np

import concourse.bacc as bacc
import concourse.tile as tile
from concourse import mybir
from concourse.bass_utils import run_bass_kernel_spmd

F32 = mybir.dt.float32
BF16 = mybir.dt.bfloat16
FP8 = mybir.dt.float8e4
DR = mybir.MatmulPerfMode.DoubleRow
AF = mybir.ActivationFunctionType
EPS = np.float32(1e-5)
N_CORES = 8
B_TOTAL = 8192
B_CORE = B_TOTAL // N_CORES
CHUNK = 512
N_CHUNKS = B_CORE // CHUNK

_f8 = lambda a: np.ascontiguousarray(a.astype(ml_dtypes.float8_e4m3fn))
_bf = lambda a: np.ascontiguousarray(a.astype(ml_dtypes.bfloat16))
_f32c = lambda a: np.ascontiguousarray(a.astype(np.float32))

# wpack column offsets (fp8 [128, 4224])
_WOFF = {"w1p0": 0, "w1r8": 256, "w201": 2304, "w223": 2944, "w24r": 3584}
_MTOFF = [0, 256, 512]  # per-Mt offsets inside a 640-col pair-set (2*Mp cols)


def _sign(a):
    return np.sign(a).astype(np.float32)


def _toeplitz1(w1s):  # [6,1,5,5] -> [160,144] rows (ky, xi<32), cols (c1,xo)
    W = np.zeros((160, 144), np.float32)
    xo = np.arange(24)
    for ky in range(5):
        for kx in range(5):
            for c1 in range(6):
                W[ky * 32 + xo + kx, c1 * 24 + xo] = w1s[c1, 0, ky, kx]
    return W


def _toeplitz2(w2s):
    """[16,6,5,5] -> main [128,5,320] rows (c1,xi24 mod 128), cols (c2,xo);
    remainder (last 16 rows of each 144-block) at 16-stride: [128,320]."""
    W = np.zeros((720, 320), np.float32)
    xo = np.arange(20)
    for ky in range(5):
        for c1 in range(6):
            for kx in range(5):
                for c2 in range(16):
                    W[ky * 144 + c1 * 24 + xo + kx, c2 * 20 + xo] = w2s[c2, c1, ky, kx]
    main = np.stack([W[144 * k : 144 * k + 128] for k in range(5)], 1)  # [128,5,320]
    rem16 = np.zeros((128, 320), np.float32)  # rows 16k+r (k<5)
    for k in range(5):
        rem16[16 * k : 16 * k + 16] = W[144 * k + 128 : 144 * k + 144]
    return main, rem16


def _affine(g, b, m, v, extra_bias):
    inv = (g.astype(np.float32) / np.sqrt(v.astype(np.float32) + EPS)).astype(np.float32)
    return inv, (inv * (extra_bias.astype(np.float32) - m.astype(np.float32)) + b.astype(np.float32)).astype(np.float32)


def build_consts(inp):
    """Host-side preprocessing of all weights/BN params into device constants."""
    C = {}
    W1 = _toeplitz1(_sign(inp["conv1_w"]))
    w2main, w2r16 = _toeplitz2(_sign(inp["conv2_w"]))
    wpack = np.zeros((128, 4224), np.float32)

    def put(name, arr):  # arr [p, cols]
        o = _WOFF[name]
        wpack[0 : arr.shape[0], o : o + arr.shape[1]] = arr

    def pair(a, b):  # [128, M] + [<=128, M] -> [128, 2M] interleaved pair-major
        out = np.zeros((128, 2, a.shape[1]), np.float32)
        out[:, 0, :] = a
        out[0 : b.shape[0], 1, :] = b
        return out.reshape(128, -1)

    put("w1p0", pair(W1[0:128, 0:128], W1[128:160, 0:128]))
    # remainder as 8 zero-masked full-width DR pairs: y1%8 -> cols 16k:16k+16.
    # Full M=128 at col base 0 keeps LDW pipelined (no tile_position) and all
    # 8 accumulate into one PSUM bank (zero cols add 0).
    w1r8 = np.zeros((160, 8, 128), np.float32)
    for k in range(8):
        w1r8[:, k, 16 * k : 16 * k + 16] = W1[:, 128:144]
    for k in range(8):
        wpack[:, _WOFF["w1r8"] + 256 * k : _WOFF["w1r8"] + 256 * k + 256] = \
            pair(w1r8[0:128, k], w1r8[128:160, k])
    for Mt in range(3):
        Mp = 128 if Mt < 2 else 64
        ms = slice(128 * Mt, 128 * Mt + Mp)
        wpack[:, _WOFF["w201"] + _MTOFF[Mt] : _WOFF["w201"] + _MTOFF[Mt] + 2 * Mp] = \
            pair(w2main[:, 0, ms], w2main[:, 1, ms])
        wpack[:, _WOFF["w223"] + _MTOFF[Mt] : _WOFF["w223"] + _MTOFF[Mt] + 2 * Mp] = \
            pair(w2main[:, 2, ms], w2main[:, 3, ms])
        wpack[:, _WOFF["w24r"] + _MTOFF[Mt] : _WOFF["w24r"] + _MTOFF[Mt] + 2 * Mp] = \
            pair(w2main[:, 4, ms], w2r16[:, ms])
    C["wpack1"] = _f8(wpack[:, 0:2304])  # conv1 weights (early load)
    C["wpack2"] = _f8(wpack[:, 2304:4224])

    # fc1, permuted to the on-chip act2 layout: block b=(Mt*20+y2), row p -> feature
    # f2 = 320*y2 + 128*Mt + p  (= c2*20+xo within the y-block), orig col c2*400+y2*20+xo
    w3s = _sign(inp["fc1_w"])  # [120, 6400]
    W3Tp = np.zeros((128, 60, 128), np.float32)  # M padded 120->128 (DR stride % 16)
    for Mt in range(3):
        Kj = 128 if Mt < 2 else 64
        m = np.arange(Kj) + 128 * Mt
        c2, xo = m // 20, m % 20
        for y2 in range(20):
            cols = c2 * 400 + y2 * 20 + xo
            W3Tp[:Kj, Mt * 20 + y2, 0:120] = w3s[:, cols].T
    C["w3t"] = _f8(W3Tp)
    w45 = np.zeros((120, 94), np.float32)
    w45[0:120, 0:84] = _sign(inp["fc2_w"]).T
    w45[0:84, 84:94] = _sign(inp["fc3_w"]).T
    C["w45"] = _bf(w45)

    s1, b1 = _affine(inp["bn1_g"], inp["bn1_b"], inp["bn1_m"], inp["bn1_v"], inp["conv1_b"])
    s2, b2 = _affine(inp["bn2_g"], inp["bn2_b"], inp["bn2_m"], inp["bn2_v"], inp["conv2_b"])
    s3, b3 = _affine(inp["bnf1_g"], inp["bnf1_b"], inp["bnf1_m"], inp["bnf1_v"], inp["fc1_b"])
    s4, b4 = _affine(inp["bnf2_g"], inp["bnf2_b"], inp["bnf2_m"], inp["bnf2_v"], inp["fc2_b"])
    s5, b5 = _affine(inp["bnf3_g"], inp["bnf3_b"], inp["bnf3_m"], inp["bnf3_v"], inp["fc3_b"])
    c1v = np.arange(144) // 24
    sc1f, bi1f = s1[c1v], b1[c1v]
    c2v = np.arange(320) // 20
    sc2f, bi2f = s2[c2v], b2[c2v]
    scp = np.zeros((128, 16), np.float32)
    scp[:, 0], scp[:, 1] = sc1f[:128], bi1f[:128]
    for k in range(8):  # remainder scale/bias at 16-stride
        scp[16 * k : 16 * k + 16, 2] = sc1f[128:]
        scp[16 * k : 16 * k + 16, 3] = bi1f[128:]
    scp[:, 4], scp[:, 5] = sc2f[0:128], bi2f[0:128]
    scp[:, 6], scp[:, 7] = sc2f[128:256], bi2f[128:256]
    scp[0:64, 8], scp[0:64, 9] = sc2f[256:320], bi2f[256:320]
    scp[0:120, 10], scp[0:120, 11] = s3, b3
    scp[0:84, 12], scp[0:84, 13] = s4, b4
    scp[0:10, 14], scp[0:10, 15] = s5, b5
    C["scp"] = _f32c(scp)
    return C


def prep_x(x):
    """sign + feature-major layout + 4 phase shifts: [B,1,28,28] ->
    per-core [4, 128, 7, B_CORE] fp8 (xT row 32y+x = sign(img[y,x]), x<28)."""
    xs = np.sign(x.reshape(B_TOTAL, 28, 28)).astype(np.float32)
    res = []
    for i in range(N_CORES):
        xc = xs[i * B_CORE : (i + 1) * B_CORE]  # [b, 28, 28]
        tmp = np.zeros((B_CORE, 28, 32), np.float32)
        tmp[:, :, 0:28] = xc
        xT = np.zeros((1024, B_CORE), np.float32)
        xT[0:896] = tmp.reshape(B_CORE, 896).T
        xq = np.stack([xT[32 * q : 32 * q + 896].reshape(7, 128, B_CORE).transpose(1, 0, 2)
                       for q in range(4)])  # [4,128,7,b]
        res.append(_f8(xq))
    return res


def build_nc(consts, b_core=B_CORE, chunk=CHUNK, stage=99):
    n_chunks = b_core // chunk
    assert chunk % 128 == 0
    nc = bacc.Bacc(None, target_bir_lowering=False, debug=False)
    xt_in = nc.declare_dram_parameter("xt", [4, 128, 7, b_core], FP8, isOutput=False)
    if stage >= 37:
        out = nc.declare_dram_parameter("out", [10, b_core], F32, isOutput=True)
    else:
        dbg = nc.declare_dram_parameter("dbg", [128, 512], F32, isOutput=True)
    dr = {k: nc.inline_tensor(v, name=f"c_{k}") for k, v in consts.items()}

    with tile.TileContext(nc) as tc, ExitStack() as ctx:
        cp = ctx.enter_context(tc.tile_pool(name="consts", bufs=1))
        xtpool = ctx.enter_context(tc.tile_pool(name="xtpool", bufs=2))
        tpp = ctx.enter_context(tc.tile_pool(name="tp", bufs=2, space="PSUM"))
        cps = ctx.enter_context(tc.tile_pool(name="cps", bufs=2, space="PSUM"))
        fcp = ctx.enter_context(tc.tile_pool(name="fcp", bufs=1, space="PSUM"))
        apool = ctx.enter_context(tc.tile_pool(name="apool", bufs=2))
        a2pool = ctx.enter_context(tc.tile_pool(name="a2pool", bufs=2))
        fpool = ctx.enter_context(tc.tile_pool(name="fpool", bufs=2))
        dpool = ctx.enter_context(tc.tile_pool(name="dpool", bufs=2))

        def load_x(c, split=False):
            xtq = [xtpool.tile([128, 7, chunk], FP8, tag=f"xt{q}", name=f"xt{q}")
                   for q in range(4)]
            halves = [(0, 5), (5, 7)] if split else [(0, 7)]
            for lo, hi in halves:
                for q in range(4):
                    nc.sync.dma_start(out=xtq[q][:, lo:hi, :],
                                      in_=xt_in[q, :, lo:hi, c * chunk : (c + 1) * chunk])
            return xtq

        def cload(name, shape, dtype=FP8):
            t = cp.tile(shape, dtype, tag=f"c_{name}", name=f"c_{name}")
            nc.sync.dma_start(out=t[:], in_=dr[name][:])
            return t

        # issue order: first input slots -> conv1 weights/scales -> the rest
        xtq_next = [xtpool.tile([128, 7, chunk], FP8, tag=f"xt{q}", name=f"xt{q}")
                    for q in range(4)]
        for q in range(4):
            nc.sync.dma_start(out=xtq_next[q][:, 0:3, :], in_=xt_in[q, :, 0:3, 0:chunk])
        wp = cp.tile([128, 4224], FP8, tag="c_wpack", name="c_wpack")
        nc.sync.dma_start(out=wp[:, 0:2304], in_=dr["wpack1"][:])
        scp = cload("scp", [128, 16], F32)
        for q in range(4):
            nc.sync.dma_start(out=xtq_next[q][:, 3:7, :], in_=xt_in[q, :, 3:7, 0:chunk])
        nc.sync.dma_start(out=wp[:, 2304:4224], in_=dr["wpack2"][:])
        w3t = cload("w3t", [128, 60, 128])
        w45 = cload("w45", [120, 94], BF16)

        # HAM warm-up burst: dep-free matmuls fill the input-DMA shadow so the
        # PE clock reaches 2.4 GHz before conv1's first real matmul.
        wub = cp.tile([128, 128], BF16, tag="warm")
        nc.gpsimd.memset(wub[:], 1.0)
        f1w = fcp.tile([128, CHUNK], F32, tag="f1ps")
        for _ in range(125):
            nc.tensor.matmul(f1w[:, 0:128], wub[:], wub[:], start=True, stop=True)

        def wdr(name, Mt=None, k=None):  # DoubleRow pair view [128, 2, Mp]
            if k is not None:
                o, Mp = _WOFF[name] + 256 * k, 128
            elif Mt is None:
                o, Mp = _WOFF[name], 128
            else:
                o, Mp = _WOFF[name] + _MTOFF[Mt], 128 if Mt < 2 else 64
            return wp[:, o : o + 2 * Mp].rearrange("p (a m) -> p a m", a=2)

        def wsl(name, p, cols):
            o = _WOFF[name]
            return wp[0:p, o : o + cols]

        def scb(col, p):  # (scale, bias) column pair from scp
            return scp[0:p, col : col + 1], scp[0:p, col + 1 : col + 2]

        for c in range(n_chunks):
            xtq = xtq_next
            if stage <= 1:
                dt_ = dpool.tile([128, 512], F32, tag="dbg")
                nc.vector.tensor_copy(out=dt_[:], in_=xtq[1][:, 0, 0:512])
                nc.sync.dma_start(out=dbg[:], in_=dt_[:])
                continue

            if c > 0:  # keep PE busy across the boundary act-wait (HAM MID)
                for _ in range(4):
                    nc.tensor.matmul(f1w[:, 0:128], wub[:], wub[:], start=True, stop=True)
            # ---- conv1: 1 DoubleRow matmul per (y1, Mt-main); the 16-feature
            # remainder accumulates into c1r at column offset 16*(y1%8) (odd
            # offsets via the zero-left-half 32-wide weight), one act per 8 y1.
            # actc slots: 0..23 main y rows; 24+3p+s = remainder bundles.
            actc = apool.tile([128, 48, chunk], FP8, tag="actc")
            for yga in range(0, 12, 2):
                if yga % 4 == 0:
                    c1r = fcp.tile([128, chunk], F32, tag="c1r")
                ps0s = []
                for yg in (yga, yga + 1):  # 4 DR mains grouped (one perf-mode run)
                    ps0 = cps.tile([128, 2, chunk], F32, tag="cps")
                    ps0s.append(ps0)
                    for ty in range(2):
                        y1 = 2 * yg + ty
                        q, t = y1 % 4, y1 // 4
                        nc.tensor.matmul(ps0[:, ty, :], wdr("w1p0"), xtq[q][:, t : t + 2, :],
                                         start=True, stop=True, perf_mode=DR)
                for yg in (yga, yga + 1):  # remainder: 1 DR matmul per y1
                    for ty in range(2):
                        y1 = 2 * yg + ty
                        k = y1 % 8
                        q, t = y1 % 4, y1 // 4
                        nc.tensor.matmul(c1r[:], wdr("w1r8", k=k), xtq[q][:, t : t + 2, :],
                                         start=(k == 0), stop=(k == 7), perf_mode=DR)
                s0, b0 = scb(0, 128)
                nc.scalar.activation(actc[:, 2 * yga : 2 * yga + 2, :], ps0s[0][:], AF.Sign,
                                     bias=b0, scale=s0)
                nc.scalar.activation(actc[:, 2 * yga + 2 : 2 * yga + 4, :], ps0s[1][:], AF.Sign,
                                     bias=b0, scale=s0)
                if yga % 4 == 2:
                    s1_, b1_ = scb(2, 128)
                    nc.scalar.activation(actc[:, 24 + yga // 4, :], c1r[:], AF.Sign,
                                         bias=b1_, scale=s1_)
                if c > 0 and yga <= 4:  # bridge early act-ring waits (HAM MID)
                    for _ in range(2):
                        nc.tensor.matmul(f1w[:, 0:128], wub[:], wub[:], start=True, stop=True)
            # 7 phase-shifted copies of the remainder bundles (16-row shifts)
            for p in range(1, 8):
                ns = 3 if p <= 3 else 2
                if p <= 3:
                    nc.vector.memset(actc[:, 24 + 3 * p + 2, :], 0.0)
                nc.sync.dma_start(out=actc[0 : 128 - 16 * p, 24 + 3 * p : 24 + 3 * p + ns, :],
                                  in_=actc[16 * p : 128, 24 : 24 + ns, :])
                nc.sync.dma_start(out=actc[128 - 16 * p : 128, 24 + 3 * p : 24 + 3 * p + 2, :],
                                  in_=actc[0 : 16 * p, 25 : 27, :])
            if c + 1 < n_chunks:  # issue next chunk's input loads early
                xtq_next = load_x(c + 1)
            if stage <= 2:
                dt_ = dpool.tile([128, 512], F32, tag="dbg")
                nc.vector.tensor_copy(out=dt_[:], in_=actc[:, 0, 0:512])
                nc.sync.dma_start(out=dbg[:], in_=dt_[:])
                continue

            # ---- conv2 (Toeplitz over actc): THREE DR matmuls per (y2, Mt);
            # fc1's 30 accumulating DR matmuls interleave as act2 slots land ----
            act2 = a2pool.tile([128, 3, 20, chunk], FP8, tag="act2")
            PAIRS = [(0, 8), (1, 9), (2, 10), (3, 11), (4, 12), (5, 13),
                     (6, 14), (7, 15), (16, 18), (17, 19)]
            for pi, (ya, yb) in enumerate(PAIRS):
                pss = []
                for Mt in range(3):
                    Mp = 128 if Mt < 2 else 64
                    ps = cps.tile([Mp, 2, chunk], F32, tag="cps", name=f"c2ps{Mt}")
                    pss.append((ps, Mp))
                    for ty, y2 in ((0, ya), (1, yb)):
                        p8, s8 = y2 % 8, y2 // 8
                        d = (24 + 3 * p8 + s8) - (y2 + 4)
                        nc.tensor.matmul(ps[0:Mp, ty, :], wdr("w201", Mt),
                                         actc[:, y2 : y2 + 2, :],
                                         start=True, stop=False, perf_mode=DR)
                        nc.tensor.matmul(ps[0:Mp, ty, :], wdr("w223", Mt),
                                         actc[:, y2 + 2 : y2 + 4, :],
                                         start=False, stop=False, perf_mode=DR)
                        nc.tensor.matmul(ps[0:Mp, ty, :], wdr("w24r", Mt),
                                         actc[:, y2 + 4 : y2 + 5 + d : d, :],
                                         start=False, stop=True, perf_mode=DR)
                for Mt, (ps, Mp) in enumerate(pss):
                    s2_, b2_ = scb(4 + 2 * Mt, Mp)
                    nc.scalar.activation(act2[0:Mp, Mt, ya : yb + 1 : yb - ya, :], ps[:],
                                         AF.Sign, bias=b2_, scale=s2_)
            if stage <= 3:
                dt_ = dpool.tile([128, 512], F32, tag="dbg")
                nc.vector.tensor_copy(out=dt_[:], in_=act2[:, 0, 0, 0:512])
                nc.sync.dma_start(out=dbg[:], in_=dt_[:])
                continue

            f1ps = fcp.tile([128, chunk], F32, tag="f1ps")
            for Mt in range(3):
                Kj = 128 if Mt < 2 else 64
                for y2 in range(0, 20, 2):
                    b = Mt * 20 + y2
                    nc.tensor.matmul(f1ps[:], w3t[0:Kj, b : b + 2, :],
                                     act2[0:Kj, Mt, y2 : y2 + 2, :],
                                     start=(b == 0), stop=(b == 58), perf_mode=DR)
            a3 = fpool.tile([120, chunk], BF16, tag="a3")
            s3_, b3_ = scb(10, 120)
            nc.scalar.activation(a3[:], f1ps[0:120, :], AF.Sign, bias=b3_, scale=s3_)
            if stage <= 35:
                dt_ = dpool.tile([128, 512], F32, tag="dbg")
                nc.any.memset(dt_[:], 0.0)
                nc.vector.tensor_copy(out=dt_[0:120, :], in_=a3[:, 0:512])
                nc.sync.dma_start(out=dbg[:], in_=dt_[:])
                continue
            f2ps = tpp.tile([84, chunk], F32, tag="tp")
            nc.tensor.matmul(f2ps[:], w45[0:120, 0:84], a3[:], start=True, stop=True)
            a4 = fpool.tile([84, chunk], BF16, tag="a4")
            s4_, b4_ = scb(12, 84)
            nc.scalar.activation(a4[:], f2ps[:], AF.Sign, bias=b4_, scale=s4_)
            f3ps = tpp.tile([10, chunk], F32, tag="tp")
            nc.tensor.matmul(f3ps[:], w45[0:84, 84:94], a4[:], start=True, stop=True)
            o5 = fpool.tile([10, chunk], F32, tag="o5")
            s5_, b5_ = scb(14, 10)
            nc.vector.tensor_scalar(o5[:], f3ps[:], s5_, b5_,
                                    mybir.AluOpType.mult, mybir.AluOpType.add)
            if stage <= 36:
                dt_ = dpool.tile([128, 512], F32, tag="dbg")
                nc.any.memset(dt_[:], 0.0)
                nc.vector.tensor_copy(out=dt_[0:10, :], in_=o5[:, 0:512])
                nc.sync.dma_start(out=dbg[:], in_=dt_[:])
                continue
            nc.sync.dma_start(out=out[:, c * chunk : (c + 1) * chunk], in_=o5[:])

    nc.compile()
    return nc


def kernel(**inputs):
    inputs = {k: np.asarray(v) for k, v in inputs.items()}
    consts = build_consts(inputs)
    nc = build_nc(consts)
    xs = prep_x(inputs["x"].astype(np.float32))
    in_maps = [{"xt": xs[i]} for i in range(N_CORES)]
    res = run_bass_kernel_spmd(nc, in_maps, core_ids=list(range(N_CORES)))
    out = np.concatenate([np.asarray(r["out"]).astype(np.float32).T for r in res.results], axis=0)
    return out.astype(np.float32)



# revision 2
# speedup vs baseline: 1.1496x; 1.1496x over previous
"""Binarized LeNet5+BN forward on 8 Trainium2 NeuronCores.

Strategy (data-parallel over batch, 1024 images/core), v2:
  * Feature-major on-chip layout; every layer = matmul-accumulate into PSUM
    followed by ONE activation op (fused conv-bias+BN+hardtanh+binarize).
  * All conv/fc1 matmul operands fp8e4 with perf_mode=DoubleRow (2 K-tiles
    per N=512 pass); host-built Toeplitz +-1/0 weights.
  * v2 pass-count cuts vs v1 (260 -> 217 passes/chunk):
    - conv1 16-feature remainders grouped: 8 y1-rows' remainders packed
      block-diagonally into ONE PSUM bank covering a 384-input-row window
      (1 DR + 1 plain pass per group of 8, x3 groups) instead of 24 passes.
    - conv2 Mt=2 (64-feature) output tiles merged across adjacent y2 pairs
      into full 128-wide passes (4 passes/pair vs 6).
    - fc1 contracts 50 full 128-row act2 tiles = 25 DR passes (vs 30).
  * Activation work split across BOTH ScalarE and VectorE (v1: scalar-only
    at 67% busy was near-critical):
    - Scalar (AF.Sign, +-1 fp8): conv1 mains+remainders, conv2-Mt2-merged,
      fc1, fc2 outputs.
    - Vector (tensor_scalar is_ge, {0,1} fp8): conv2 Mt0/Mt1 outputs.
      Consumers fold the {0,1} encoding: fc1 weight rows for those features
      are 2*w*sign(s2), and the -sum(w*d) constant folds into fc1's Sign
      bias (b3' = b3 - s3*c3). Exact in fp8.
  * Weights packed into few DMA loads issued after the first input tiles;
    HAM warm-up burst before conv1; double-buffered pools throughout.
"""

from contextlib import ExitStack

import ml_dtypes
import numpy as np

import concourse.bacc as bacc
import concourse.tile as tile
from concourse import mybir
from concourse.bass_utils import run_bass_kernel_spmd

F32 = mybir.dt.float32
BF16 = mybir.dt.bfloat16
FP8 = mybir.dt.float8e4
DR = mybir.MatmulPerfMode.DoubleRow
AF = mybir.ActivationFunctionType
GE = mybir.AluOpType.is_ge
EPS = np.float32(1e-5)
N_CORES = 8
B_TOTAL = 8192
B_CORE = B_TOTAL // N_CORES
CHUNK = 512
N_CHUNKS = B_CORE // CHUNK

_f8 = lambda a: np.ascontiguousarray(a.astype(ml_dtypes.float8_e4m3fn))
_bf = lambda a: np.ascontiguousarray(a.astype(ml_dtypes.bfloat16))
_f32c = lambda a: np.ascontiguousarray(a.astype(np.float32))

# wpack column offsets (fp8 [128, 3968]); split for early conv1 load
_WOFF = {"w1p0": 0, "w1rA": 256, "w1rB": 1024, "w201": 1408, "w223": 1920,
         "w24r": 2432, "wm2": 2944, "wr2m": 3712}
WPACK_COLS = 3968
WSPLIT = 1408  # conv1 weights end


def _sign(a):
    return np.sign(a).astype(np.float32)


def _toeplitz1(w1s):  # [6,1,5,5] -> [160,144] rows (ky, xi<32), cols (c1,xo)
    W = np.zeros((160, 144), np.float32)
    xo = np.arange(24)
    for ky in range(5):
        for kx in range(5):
            for c1 in range(6):
                W[ky * 32 + xo + kx, c1 * 24 + xo] = w1s[c1, 0, ky, kx]
    return W


def _toeplitz2(w2s):
    """[16,6,5,5] -> main [128,5,320] rows (c1,xi24 mod 128), cols (c2,xo);
    remainder (last 16 rows of each 144-block) at 16-stride: [128,320]."""
    W = np.zeros((720, 320), np.float32)
    xo = np.arange(20)
    for ky in range(5):
        for c1 in range(6):
            for kx in range(5):
                for c2 in range(16):
                    W[ky * 144 + c1 * 24 + xo + kx, c2 * 20 + xo] = w2s[c2, c1, ky, kx]
    main = np.stack([W[144 * k : 144 * k + 128] for k in range(5)], 1)  # [128,5,320]
    rem16 = np.zeros((128, 320), np.float32)  # rows 16k+r (k<5)
    for k in range(5):
        rem16[16 * k : 16 * k + 16] = W[144 * k + 128 : 144 * k + 144]
    return main, rem16


def _affine(g, b, m, v, extra_bias):
    inv = (g.astype(np.float32) / np.sqrt(v.astype(np.float32) + EPS)).astype(np.float32)
    return inv, (inv * (extra_bias.astype(np.float32) - m.astype(np.float32)) + b.astype(np.float32)).astype(np.float32)


def _pair(a, b):  # [128, M] + [<=128, M] -> [128, 2M] interleaved pair-major
    out = np.zeros((128, 2, a.shape[1]), np.float32)
    out[:, 0, :] = a
    out[0 : b.shape[0], 1, :] = b
    return out.reshape(128, -1)


def build_consts(inp):
    """Host-side preprocessing of all weights/BN params into device constants."""
    C = {}
    W1 = _toeplitz1(_sign(inp["conv1_w"]))
    w2main, w2r16 = _toeplitz2(_sign(inp["conv2_w"]))
    wpack = np.zeros((128, WPACK_COLS), np.float32)

    wpack[:, 0:256] = _pair(W1[0:128, 0:128], W1[128:160, 0:128])
    # conv1 remainder groups: group g covers y1 in [8g, 8g+8); its windows
    # span input rows [256g, 256g+384). Col 16k+j <-> (y1=8g+k, feat 128+j).
    for g in range(3):
        A = np.zeros((256, 128), np.float32)
        Bm = np.zeros((128, 128), np.float32)
        for k in range(8):
            full = np.zeros((384, 16), np.float32)
            full[32 * k : 32 * k + 160, :] = W1[:, 128:144]
            A[:, 16 * k : 16 * k + 16] = full[0:256]
            Bm[:, 16 * k : 16 * k + 16] = full[256:384]
        wpack[:, _WOFF["w1rA"] + 256 * g : _WOFF["w1rA"] + 256 * g + 256] = \
            _pair(A[0:128], A[128:256])
        wpack[:, _WOFF["w1rB"] + 128 * g : _WOFF["w1rB"] + 128 * g + 128] = Bm
    # conv2 Mt0/Mt1: 3 DR pairs each (ky01, ky23, ky4+rem)
    for Mt in range(2):
        ms = slice(128 * Mt, 128 * Mt + 128)
        wpack[:, _WOFF["w201"] + 256 * Mt : _WOFF["w201"] + 256 * Mt + 256] = \
            _pair(w2main[:, 0, ms], w2main[:, 1, ms])
        wpack[:, _WOFF["w223"] + 256 * Mt : _WOFF["w223"] + 256 * Mt + 256] = \
            _pair(w2main[:, 2, ms], w2main[:, 3, ms])
        wpack[:, _WOFF["w24r"] + 256 * Mt : _WOFF["w24r"] + 256 * Mt + 256] = \
            _pair(w2main[:, 4, ms], w2r16[:, ms])
    # conv2 Mt2 merged across adjacent (ya, yb=ya+1): cols 0:64 <- ya feats
    # 256:320, cols 64:128 <- yb. Main pass j contracts slots (ya+2j, ya+2j+1).
    m2 = slice(256, 320)
    for j in range(3):
        blk = np.zeros((128, 2, 128), np.float32)
        for a in range(2):
            so = 2 * j + a  # slot offset rel. ya
            if so <= 4:
                blk[:, a, 0:64] = w2main[:, so, m2]
            if 0 <= so - 1 <= 4:
                blk[:, a, 64:128] = w2main[:, so - 1, m2]
        wpack[:, _WOFF["wm2"] + 256 * j : _WOFF["wm2"] + 256 * j + 256] = \
            blk.reshape(128, 256)
    blk = np.zeros((128, 2, 128), np.float32)
    blk[:, 0, 0:64] = w2r16[:, m2]   # pair elem 0 = ya's rem bundle slot
    blk[:, 1, 64:128] = w2r16[:, m2]  # pair elem 1 = yb's rem bundle slot
    wpack[:, _WOFF["wr2m"] : _WOFF["wr2m"] + 256] = blk.reshape(128, 256)

    C["wpack1"] = _f8(wpack[:, 0:WSPLIT])
    C["wpack2"] = _f8(wpack[:, WSPLIT:WPACK_COLS])

    # BN affine folds
    s1, b1 = _affine(inp["bn1_g"], inp["bn1_b"], inp["bn1_m"], inp["bn1_v"], inp["conv1_b"])
    s2, b2 = _affine(inp["bn2_g"], inp["bn2_b"], inp["bn2_m"], inp["bn2_v"], inp["conv2_b"])
    s3, b3 = _affine(inp["bnf1_g"], inp["bnf1_b"], inp["bnf1_m"], inp["bnf1_v"], inp["fc1_b"])
    s4, b4 = _affine(inp["bnf2_g"], inp["bnf2_b"], inp["bnf2_m"], inp["bnf2_v"], inp["fc2_b"])
    s5, b5 = _affine(inp["bnf3_g"], inp["bnf3_b"], inp["bnf3_m"], inp["bnf3_v"], inp["fc3_b"])
    c1v = np.arange(144) // 24
    sc1f, bi1f = s1[c1v], b1[c1v]
    c2v = np.arange(320) // 20
    sc2f, bi2f = s2[c2v], b2[c2v]

    # fc1, permuted to on-chip act2 layout [128, 50, 128] (50 full K-tiles):
    # pair-group p (ya=2p, yb=2p+1) owns blocks 5p..5p+4:
    #   5p+0: ya feats 0:128 ({0,1} DVE) | 5p+1: yb feats 0:128
    #   5p+2: ya feats 128:256           | 5p+3: yb feats 128:256
    #   5p+4: [ya feats 256:320 | yb feats 256:320] (+-1 scalar)
    # {0,1} rows get w'' = 2*d*w (d = sign(s2) flip); fold c3 into fc1 bias.
    w3s = _sign(inp["fc1_w"])  # [120, 6400]
    d2f = np.where(sc2f >= 0, np.float32(1.0), np.float32(-1.0))

    def cols(y2, m):
        return (m // 20) * 400 + y2 * 20 + (m % 20)

    W3T = np.zeros((128, 50, 128), np.float32)
    for p in range(10):
        ya, yb = 2 * p, 2 * p + 1
        m0 = np.arange(128)
        m1 = np.arange(128) + 128
        mm2 = np.arange(64) + 256
        W3T[:, 5 * p + 0, 0:120] = (2 * d2f[m0])[:, None] * w3s[:, cols(ya, m0)].T
        W3T[:, 5 * p + 1, 0:120] = (2 * d2f[m0])[:, None] * w3s[:, cols(yb, m0)].T
        W3T[:, 5 * p + 2, 0:120] = (2 * d2f[m1])[:, None] * w3s[:, cols(ya, m1)].T
        W3T[:, 5 * p + 3, 0:120] = (2 * d2f[m1])[:, None] * w3s[:, cols(yb, m1)].T
        W3T[0:64, 5 * p + 4, 0:120] = w3s[:, cols(ya, mm2)].T
        W3T[64:128, 5 * p + 4, 0:120] = w3s[:, cols(yb, mm2)].T
    C["w3t"] = _f8(W3T)
    # fold constant: c3[m] = sum over {0,1}-encoded inputs of w3s*d
    mdv = np.arange(256)
    c3 = np.zeros(120, np.float32)
    for y2 in range(20):
        c3 += (w3s[:, cols(y2, mdv)] * d2f[mdv][None, :]).sum(1)

    w45 = np.zeros((120, 94), np.float32)
    w45[0:120, 0:84] = _sign(inp["fc2_w"]).T
    w45[0:84, 84:94] = _sign(inp["fc3_w"]).T
    C["w45"] = _bf(w45)

    scp = np.zeros((128, 16), np.float32)
    scp[:, 0], scp[:, 1] = sc1f[:128], bi1f[:128]
    for k in range(8):  # remainder scale/bias at 16-stride
        scp[16 * k : 16 * k + 16, 2] = sc1f[128:]
        scp[16 * k : 16 * k + 16, 3] = bi1f[128:]
    scp[:, 4] = -bi2f[0:128] / sc2f[0:128]        # DVE thr Mt0
    scp[:, 5] = -bi2f[128:256] / sc2f[128:256]    # DVE thr Mt1
    scp[0:64, 6], scp[64:128, 6] = sc2f[256:320], sc2f[256:320]
    scp[0:64, 7], scp[64:128, 7] = bi2f[256:320], bi2f[256:320]
    scp[0:120, 8], scp[0:120, 9] = s3, b3 - s3 * c3
    scp[0:84, 10], scp[0:84, 11] = s4, b4
    scp[0:10, 12], scp[0:10, 13] = s5, b5
    C["scp"] = _f32c(scp)
    return C


def prep_x(x):
    """sign + feature-major layout + 4 phase shifts: [B,1,28,28] ->
    per-core [4, 128, 7, B_CORE] fp8 (xT row 32y+x = sign(img[y,x]), x<28)."""
    xs = np.sign(x.reshape(B_TOTAL, 28, 28)).astype(np.float32)
    res = []
    for i in range(N_CORES):
        xc = xs[i * B_CORE : (i + 1) * B_CORE]  # [b, 28, 28]
        tmp = np.zeros((B_CORE, 28, 32), np.float32)
        tmp[:, :, 0:28] = xc
        xT = np.zeros((1024, B_CORE), np.float32)
        xT[0:896] = tmp.reshape(B_CORE, 896).T
        xq = np.stack([xT[32 * q : 32 * q + 896].reshape(7, 128, B_CORE).transpose(1, 0, 2)
                       for q in range(4)])  # [4,128,7,b]
        res.append(_f8(xq))
    return res


def build_nc(consts, b_core=B_CORE, chunk=CHUNK, stage=99):
    n_chunks = b_core // chunk
    assert chunk % 128 == 0
    nc = bacc.Bacc(None, target_bir_lowering=False, debug=False)
    xt_in = nc.declare_dram_parameter("xt", [4, 128, 7, b_core], FP8, isOutput=False)
    if stage >= 37:
        out = nc.declare_dram_parameter("out", [10, b_core], F32, isOutput=True)
    else:
        dbg = nc.declare_dram_parameter("dbg", [128, 512], F32, isOutput=True)
    dr = {k: nc.inline_tensor(v, name=f"c_{k}") for k, v in consts.items()}

    with tile.TileContext(nc) as tc, ExitStack() as ctx:
        cp = ctx.enter_context(tc.tile_pool(name="consts", bufs=1))
        xtpool = ctx.enter_context(tc.tile_pool(name="xtpool", bufs=2))
        tpp = ctx.enter_context(tc.tile_pool(name="tp", bufs=2, space="PSUM"))
        cps = ctx.enter_context(tc.tile_pool(name="cps", bufs=2, space="PSUM"))
        fcp = ctx.enter_context(tc.tile_pool(name="fcp", bufs=1, space="PSUM"))
        apool = ctx.enter_context(tc.tile_pool(name="apool", bufs=2))
        a2pool = ctx.enter_context(tc.tile_pool(name="a2pool", bufs=2))
        fpool = ctx.enter_context(tc.tile_pool(name="fpool", bufs=2))
        dpool = ctx.enter_context(tc.tile_pool(name="dpool", bufs=2))

        def load_x(c):
            xtq = [xtpool.tile([128, 7, chunk], FP8, tag=f"xt{q}", name=f"xt{q}")
                   for q in range(4)]
            for q in range(4):
                nc.sync.dma_start(out=xtq[q][:], in_=xt_in[q, :, :, c * chunk : (c + 1) * chunk])
            return xtq

        def cload(name, shape, dtype=FP8):
            t = cp.tile(shape, dtype, tag=f"c_{name}", name=f"c_{name}")
            nc.sync.dma_start(out=t[:], in_=dr[name][:])
            return t

        # issue order: first input slots -> conv1 weights/scales -> the rest
        xtq_next = [xtpool.tile([128, 7, chunk], FP8, tag=f"xt{q}", name=f"xt{q}")
                    for q in range(4)]
        for q in range(4):
            nc.sync.dma_start(out=xtq_next[q][:, 0:3, :], in_=xt_in[q, :, 0:3, 0:chunk])
        wp = cp.tile([128, WPACK_COLS], FP8, tag="c_wpack", name="c_wpack")
        nc.sync.dma_start(out=wp[:, 0:WSPLIT], in_=dr["wpack1"][:])
        scp = cload("scp", [128, 16], F32)
        for q in range(4):
            nc.sync.dma_start(out=xtq_next[q][:, 3:7, :], in_=xt_in[q, :, 3:7, 0:chunk])
        nc.sync.dma_start(out=wp[:, WSPLIT:WPACK_COLS], in_=dr["wpack2"][:])
        w3t = cload("w3t", [128, 50, 128])
        w45 = cload("w45", [120, 94], BF16)

        # HAM warm-up burst: dep-free matmuls fill the input-DMA shadow so the
        # PE clock reaches 2.4 GHz before conv1's first real matmul.
        wub = cp.tile([128, 128], BF16, tag="warm")
        nc.gpsimd.memset(wub[:], 1.0)
        f1w = fcp.tile([128, CHUNK], F32, tag="f1ps")
        for _ in range(125):
            nc.tensor.matmul(f1w[:, 0:128], wub[:], wub[:], start=True, stop=True)

        def wdr(name, Mt=None, g=None, j=None):  # DoubleRow pair view [128, 2, 128]
            o = _WOFF[name]
            if Mt is not None:
                o += 256 * Mt
            if g is not None:
                o += 256 * g
            if j is not None:
                o += 256 * j
            return wp[:, o : o + 256].rearrange("p (a m) -> p a m", a=2)

        def scb(col, p):  # (scale, bias) column pair from scp
            return scp[0:p, col : col + 1], scp[0:p, col + 1 : col + 2]

        for c in range(n_chunks):
            xtq = xtq_next
            if stage <= 1:
                dt_ = dpool.tile([128, 512], F32, tag="dbg")
                nc.vector.tensor_copy(out=dt_[:], in_=xtq[1][:, 0, 0:512])
                nc.sync.dma_start(out=dbg[:], in_=dt_[:])
                continue

            if c > 0:  # keep PE busy across the boundary act-wait (HAM MID)
                for _ in range(4):
                    nc.tensor.matmul(f1w[:, 0:128], wub[:], wub[:], start=True, stop=True)
            # ---- conv1: 1 DoubleRow matmul per (y1, main); remainders of 8 y1
            # grouped block-diagonally into one PSUM bank (1 DR + 1 plain pass
            # per group), output at 16-feature stride = bundle layout.
            # actc slots: 0..23 main y rows; 24+3p+s = remainder bundles.
            actc = apool.tile([128, 48, chunk], FP8, tag="actc")
            for yga in range(0, 12, 2):
                ps0s = []
                for yg in (yga, yga + 1):  # 4 DR mains grouped
                    ps0 = cps.tile([128, 2, chunk], F32, tag="cps")
                    ps0s.append(ps0)
                    for ty in range(2):
                        y1 = 2 * yg + ty
                        q, t = y1 % 4, y1 // 4
                        nc.tensor.matmul(ps0[:, ty, :], wdr("w1p0"), xtq[q][:, t : t + 2, :],
                                         start=True, stop=True, perf_mode=DR)
                if yga <= 4:  # remainder group g: 2 passes
                    g = yga // 2
                    c1r = fcp.tile([128, chunk], F32, tag="c1r")
                    nc.tensor.matmul(c1r[:], wdr("w1rA", g=g), xtq[0][:, 2 * g : 2 * g + 2, :],
                                     start=True, stop=False, perf_mode=DR)
                    nc.tensor.matmul(c1r[:], wp[:, _WOFF["w1rB"] + 128 * g : _WOFF["w1rB"] + 128 * g + 128],
                                     xtq[0][:, 2 * g + 2, :], start=False, stop=True)
                s0, b0 = scb(0, 128)
                nc.scalar.activation(actc[:, 2 * yga : 2 * yga + 2, :], ps0s[0][:], AF.Sign,
                                     bias=b0, scale=s0)
                nc.scalar.activation(actc[:, 2 * yga + 2 : 2 * yga + 4, :], ps0s[1][:], AF.Sign,
                                     bias=b0, scale=s0)
                if yga <= 4:
                    s1_, b1_ = scb(2, 128)
                    nc.scalar.activation(actc[:, 24 + yga // 2, :], c1r[:], AF.Sign,
                                         bias=b1_, scale=s1_)
                if c > 0 and yga <= 4:  # bridge early act-ring waits (HAM MID)
                    for _ in range(2):
                        nc.tensor.matmul(f1w[:, 0:128], wub[:], wub[:], start=True, stop=True)
            # 7 phase-shifted copies of the remainder bundles (16-row shifts)
            for p in range(1, 8):
                ns = 3 if p <= 3 else 2
                if p <= 3:
                    nc.vector.memset(actc[:, 24 + 3 * p + 2, :], 0.0)
                nc.sync.dma_start(out=actc[0 : 128 - 16 * p, 24 + 3 * p : 24 + 3 * p + ns, :],
                                  in_=actc[16 * p : 128, 24 : 24 + ns, :])
                nc.sync.dma_start(out=actc[128 - 16 * p : 128, 24 + 3 * p : 24 + 3 * p + 2, :],
                                  in_=actc[0 : 16 * p, 25 : 27, :])
            if c + 1 < n_chunks:  # issue next chunk's input loads early
                xtq_next = load_x(c + 1)
            if stage <= 2:
                dt_ = dpool.tile([128, 512], F32, tag="dbg")
                nc.vector.tensor_copy(out=dt_[:], in_=actc[:, 0, 0:512])
                nc.sync.dma_start(out=dbg[:], in_=dt_[:])
                continue

            # ---- conv2 over adjacent pairs (ya=2p, yb=ya+1) ----
            # act2 [128, 50, chunk]: p owns slots 5p..5p+4 (see build_consts).
            act2 = a2pool.tile([128, 50, chunk], FP8, tag="act2")
            for Mt in range(2):  # Mt0/Mt1: 3 DR passes per y2; DVE is_ge act
                for p in range(10):
                    ya = 2 * p
                    ps = cps.tile([128, 2, chunk], F32, tag="cps", name=f"c2ps{Mt}")
                    for ty, y2 in ((0, ya), (1, ya + 1)):
                        p8, s8 = y2 % 8, y2 // 8
                        d = (24 + 3 * p8 + s8) - (y2 + 4)
                        nc.tensor.matmul(ps[:, ty, :], wdr("w201", Mt=Mt),
                                         actc[:, y2 : y2 + 2, :],
                                         start=True, stop=False, perf_mode=DR)
                        nc.tensor.matmul(ps[:, ty, :], wdr("w223", Mt=Mt),
                                         actc[:, y2 + 2 : y2 + 4, :],
                                         start=False, stop=False, perf_mode=DR)
                        nc.tensor.matmul(ps[:, ty, :], wdr("w24r", Mt=Mt),
                                         actc[:, y2 + 4 : y2 + 5 + d : d, :],
                                         start=False, stop=True, perf_mode=DR)
                    nc.vector.tensor_scalar(act2[:, 5 * p + 2 * Mt : 5 * p + 2 * Mt + 2, :],
                                            ps[:], scp[0:128, 4 + Mt : 5 + Mt], None, GE)
            for p in range(10):  # Mt2 merged: 4 DR passes per pair; scalar act
                ya = 2 * p
                ps = tpp.tile([128, chunk], F32, tag="tp", name="m2ps")
                for j in range(3):
                    nc.tensor.matmul(ps[:], wdr("wm2", j=j),
                                     actc[:, ya + 2 * j : ya + 2 * j + 2, :],
                                     start=(j == 0), stop=False, perf_mode=DR)
                sa = 24 + 3 * (ya % 8) + ya // 8
                nc.tensor.matmul(ps[:], wdr("wr2m"), actc[:, sa : sa + 4 : 3, :],
                                 start=False, stop=True, perf_mode=DR)
                s2_, b2_ = scb(6, 128)
                nc.scalar.activation(act2[:, 5 * p + 4, :], ps[:], AF.Sign,
                                     bias=b2_, scale=s2_)
            if stage <= 3:
                dt_ = dpool.tile([128, 512], F32, tag="dbg")
                nc.vector.tensor_copy(out=dt_[:], in_=act2[:, 0, 0:512])
                nc.sync.dma_start(out=dbg[:], in_=dt_[:])
                continue

            # ---- fc1: 25 DR passes over 50 full K-tiles ----
            f1ps = fcp.tile([128, chunk], F32, tag="f1ps")
            k = 0
            for p in range(10):
                for off in (0, 2):
                    b = 5 * p + off
                    nc.tensor.matmul(f1ps[:], w3t[:, b : b + 2, :], act2[:, b : b + 2, :],
                                     start=(k == 0), stop=False, perf_mode=DR)
                    k += 1
            for q in range(5):
                b = 10 * q + 4
                nc.tensor.matmul(f1ps[:], w3t[:, b : b + 6 : 5, :], act2[:, b : b + 6 : 5, :],
                                 start=False, stop=(q == 4), perf_mode=DR)
            a3 = fpool.tile([120, chunk], BF16, tag="a3")
            s3_, b3_ = scb(8, 120)
            nc.scalar.activation(a3[:], f1ps[0:120, :], AF.Sign, bias=b3_, scale=s3_)
            if stage <= 35:
                dt_ = dpool.tile([128, 512], F32, tag="dbg")
                nc.any.memset(dt_[:], 0.0)
                nc.vector.tensor_copy(out=dt_[0:120, :], in_=a3[:, 0:512])
                nc.sync.dma_start(out=dbg[:], in_=dt_[:])
                continue
            f2ps = tpp.tile([84, chunk], F32, tag="tp")
            nc.tensor.matmul(f2ps[:], w45[0:120, 0:84], a3[:], start=True, stop=True)
            a4 = fpool.tile([84, chunk], BF16, tag="a4")
            s4_, b4_ = scb(10, 84)
            nc.scalar.activation(a4[:], f2ps[:], AF.Sign, bias=b4_, scale=s4_)
            f3ps = tpp.tile([10, chunk], F32, tag="tp")
            nc.tensor.matmul(f3ps[:], w45[0:84, 84:94], a4[:], start=True, stop=True)
            o5 = fpool.tile([10, chunk], F32, tag="o5")
            s5_, b5_ = scb(12, 10)
            nc.vector.tensor_scalar(o5[:], f3ps[:], s5_, b5_,
                                    mybir.AluOpType.mult, mybir.AluOpType.add)
            if stage <= 36:
                dt_ = dpool.tile([128, 512], F32, tag="dbg")
                nc.any.memset(dt_[:], 0.0)
                nc.vector.tensor_copy(out=dt_[0:10, :], in_=o5[:, 0:512])
                nc.sync.dma_start(out=dbg[:], in_=dt_[:])
                continue
            nc.sync.dma_start(out=out[:, c * chunk : (c + 1) * chunk], in_=o5[:])

    nc.compile()
    return nc


def kernel(**inputs):
    inputs = {k: np.asarray(v) for k, v in inputs.items()}
    consts = build_consts(inputs)
    nc = build_nc(consts)
    xs = prep_x(inputs["x"].astype(np.float32))
    in_maps = [{"xt": xs[i]} for i in range(N_CORES)]
    res = run_bass_kernel_spmd(nc, in_maps, core_ids=list(range(N_CORES)))
    out = np.concatenate([np.asarray(r["out"]).astype(np.float32).T for r in res.results], axis=0)
    return out.astype(np.float32)


# revision 11
# speedup vs baseline: 1.1690x; 1.0170x over previous
"""Binarized LeNet5+BN forward on 8 Trainium2 NeuronCores.

Strategy (data-parallel over batch, 1024 images/core), v2:
  * Feature-major on-chip layout; every layer = matmul-accumulate into PSUM
    followed by ONE activation op (fused conv-bias+BN+hardtanh+binarize).
  * All conv/fc1 matmul operands fp8e4 with perf_mode=DoubleRow (2 K-tiles
    per N=512 pass); host-built Toeplitz +-1/0 weights.
  * v2 pass-count cuts vs v1 (260 -> 217 passes/chunk):
    - conv1 16-feature remainders grouped: 8 y1-rows' remainders packed
      block-diagonally into ONE PSUM bank covering a 384-input-row window
      (1 DR + 1 plain pass per group of 8, x3 groups) instead of 24 passes.
    - conv2 Mt=2 (64-feature) output tiles merged across adjacent y2 pairs
      into full 128-wide passes (4 passes/pair vs 6).
    - fc1 contracts 50 full 128-row act2 tiles = 25 DR passes (vs 30).
  * Activation work split across BOTH ScalarE and VectorE (v1: scalar-only
    at 67% busy was near-critical):
    - Scalar (AF.Sign, +-1 fp8): conv1 mains+remainders, conv2-Mt2-merged,
      fc1, fc2 outputs.
    - Vector (tensor_scalar is_ge, {0,1} fp8): conv2 Mt0/Mt1 outputs.
      Consumers fold the {0,1} encoding: fc1 weight rows for those features
      are 2*w*sign(s2), and the -sum(w*d) constant folds into fc1's Sign
      bias (b3' = b3 - s3*c3). Exact in fp8.
  * Weights packed into few DMA loads issued after the first input tiles;
    HAM warm-up burst before conv1; double-buffered pools throughout.
"""

from contextlib import ExitStack

import ml_dtypes
import numpy as np

import concourse.bacc as bacc
import concourse.tile as tile
from concourse import mybir
from concourse.bass_utils import run_bass_kernel_spmd

F32 = mybir.dt.float32
BF16 = mybir.dt.bfloat16
FP8 = mybir.dt.float8e4
DR = mybir.MatmulPerfMode.DoubleRow
AF = mybir.ActivationFunctionType
GE = mybir.AluOpType.is_ge
EPS = np.float32(1e-5)
N_CORES = 8
B_TOTAL = 8192
B_CORE = B_TOTAL // N_CORES
CHUNK = 512
N_CHUNKS = B_CORE // CHUNK

_f8 = lambda a: np.ascontiguousarray(a.astype(ml_dtypes.float8_e4m3fn))
_bf = lambda a: np.ascontiguousarray(a.astype(ml_dtypes.bfloat16))
_f32c = lambda a: np.ascontiguousarray(a.astype(np.float32))

# wpack column offsets (fp8 [128, 5504]); split for early conv1 load.
# conv2 Mt0/Mt1 weights come in even/odd-y2 variants: odd actc slots are
# DVE-produced {0,1} so their rows carry 2*d1 scaling (see build_consts).
_WOFF = {"w1p0": 0, "w1rA": 256, "w1rB": 1024,
         "w201e": 1408, "w201o": 1920, "w223e": 2432, "w223o": 2944,
         "w24re": 3456, "w24ro": 3968, "wm2": 4480, "wr2m": 5248}
WPACK_COLS = 5504
WSPLIT = 1408  # conv1 weights end
N_WARM = 64  # HAM warm-up matmul count


def _sign(a):
    return np.sign(a).astype(np.float32)


def _toeplitz1(w1s):  # [6,1,5,5] -> [160,144] rows (ky, xi<32), cols (c1,xo)
    W = np.zeros((160, 144), np.float32)
    xo = np.arange(24)
    for ky in range(5):
        for kx in range(5):
            for c1 in range(6):
                W[ky * 32 + xo + kx, c1 * 24 + xo] = w1s[c1, 0, ky, kx]
    return W


def _toeplitz2(w2s):
    """[16,6,5,5] -> main [128,5,320] rows (c1,xi24 mod 128), cols (c2,xo);
    remainder (last 16 rows of each 144-block) at 16-stride: [128,320]."""
    W = np.zeros((720, 320), np.float32)
    xo = np.arange(20)
    for ky in range(5):
        for c1 in range(6):
            for kx in range(5):
                for c2 in range(16):
                    W[ky * 144 + c1 * 24 + xo + kx, c2 * 20 + xo] = w2s[c2, c1, ky, kx]
    main = np.stack([W[144 * k : 144 * k + 128] for k in range(5)], 1)  # [128,5,320]
    rem16 = np.zeros((128, 320), np.float32)  # rows 16k+r (k<5)
    for k in range(5):
        rem16[16 * k : 16 * k + 16] = W[144 * k + 128 : 144 * k + 144]
    return main, rem16


def _affine(g, b, m, v, extra_bias):
    inv = (g.astype(np.float32) / np.sqrt(v.astype(np.float32) + EPS)).astype(np.float32)
    return inv, (inv * (extra_bias.astype(np.float32) - m.astype(np.float32)) + b.astype(np.float32)).astype(np.float32)


def _pair(a, b):  # [128, M] + [<=128, M] -> [128, 2M] interleaved pair-major
    out = np.zeros((128, 2, a.shape[1]), np.float32)
    out[:, 0, :] = a
    out[0 : b.shape[0], 1, :] = b
    return out.reshape(128, -1)


def build_consts(inp):
    """Host-side preprocessing of all weights/BN params into device constants."""
    C = {}
    W1 = _toeplitz1(_sign(inp["conv1_w"]))
    w2main, w2r16 = _toeplitz2(_sign(inp["conv2_w"]))
    wpack = np.zeros((128, WPACK_COLS), np.float32)

    wpack[:, 0:256] = _pair(W1[0:128, 0:128], W1[128:160, 0:128])
    # conv1 remainder groups: group g covers y1 in [8g, 8g+8); its windows
    # span input rows [256g, 256g+384). Col 16k+j <-> (y1=8g+k, feat 128+j).
    for g in range(3):
        A = np.zeros((256, 128), np.float32)
        Bm = np.zeros((128, 128), np.float32)
        for k in range(8):
            full = np.zeros((384, 16), np.float32)
            full[32 * k : 32 * k + 160, :] = W1[:, 128:144]
            A[:, 16 * k : 16 * k + 16] = full[0:256]
            Bm[:, 16 * k : 16 * k + 16] = full[256:384]
        wpack[:, _WOFF["w1rA"] + 256 * g : _WOFF["w1rA"] + 256 * g + 256] = \
            _pair(A[0:128], A[128:256])
        wpack[:, _WOFF["w1rB"] + 128 * g : _WOFF["w1rB"] + 128 * g + 128] = Bm
    # BN affine folds (needed before conv2 packing for the d1 row scaling)
    s1, b1 = _affine(inp["bn1_g"], inp["bn1_b"], inp["bn1_m"], inp["bn1_v"], inp["conv1_b"])
    s2, b2 = _affine(inp["bn2_g"], inp["bn2_b"], inp["bn2_m"], inp["bn2_v"], inp["conv2_b"])
    s3, b3 = _affine(inp["bnf1_g"], inp["bnf1_b"], inp["bnf1_m"], inp["bnf1_v"], inp["fc1_b"])
    s4, b4 = _affine(inp["bnf2_g"], inp["bnf2_b"], inp["bnf2_m"], inp["bnf2_v"], inp["fc2_b"])
    s5, b5 = _affine(inp["bnf3_g"], inp["bnf3_b"], inp["bnf3_m"], inp["bnf3_v"], inp["fc3_b"])
    c1v = np.arange(144) // 24
    sc1f, bi1f = s1[c1v], b1[c1v]
    c2v = np.arange(320) // 20
    sc2f, bi2f = s2[c2v], b2[c2v]
    # conv1 main slots: even y1 -> ScalarE Sign (+-1); odd y1 -> VectorE is_ge
    # ({0,1}); d1 = flip for negative BN scale on the {0,1} decode.
    d1f = np.where(sc1f[0:128] >= 0, np.float32(1.0), np.float32(-1.0))

    def _rsc(w, par_odd):  # scale rows by 2*d1 when the slot parity is odd
        return (2.0 * d1f)[:, None] * w if par_odd else w

    # conv2 Mt0/Mt1: 3 DR pairs each (ky01, ky23, ky4+rem), e/o y2 variants.
    # Pair elem a of pass j touches slot y2+2j+a -> parity (y2+a) % 2.
    for Mt in range(2):
        ms = slice(128 * Mt, 128 * Mt + 128)
        for v, sfx in ((0, "e"), (1, "o")):
            wpack[:, _WOFF["w201" + sfx] + 256 * Mt : _WOFF["w201" + sfx] + 256 * Mt + 256] = \
                _pair(_rsc(w2main[:, 0, ms], (v + 0) % 2), _rsc(w2main[:, 1, ms], (v + 1) % 2))
            wpack[:, _WOFF["w223" + sfx] + 256 * Mt : _WOFF["w223" + sfx] + 256 * Mt + 256] = \
                _pair(_rsc(w2main[:, 2, ms], (v + 0) % 2), _rsc(w2main[:, 3, ms], (v + 1) % 2))
            wpack[:, _WOFF["w24r" + sfx] + 256 * Mt : _WOFF["w24r" + sfx] + 256 * Mt + 256] = \
                _pair(_rsc(w2main[:, 4, ms], (v + 0) % 2), w2r16[:, ms])
    # conv2 Mt2 merged across adjacent (ya, yb=ya+1): cols 0:64 <- ya feats
    # 256:320, cols 64:128 <- yb. Main pass j contracts slots (ya+2j, ya+2j+1);
    # elem a parity = a (ya even).
    m2 = slice(256, 320)
    for j in range(3):
        blk = np.zeros((128, 2, 128), np.float32)
        for a in range(2):
            so = 2 * j + a  # slot offset rel. ya
            if so <= 4:
                blk[:, a, 0:64] = _rsc(w2main[:, so, m2], a % 2)
            if 0 <= so - 1 <= 4:
                blk[:, a, 64:128] = _rsc(w2main[:, so - 1, m2], a % 2)
        wpack[:, _WOFF["wm2"] + 256 * j : _WOFF["wm2"] + 256 * j + 256] = \
            blk.reshape(128, 256)
    blk = np.zeros((128, 2, 128), np.float32)
    blk[:, 0, 0:64] = w2r16[:, m2]   # pair elem 0 = ya's rem bundle slot
    blk[:, 1, 64:128] = w2r16[:, m2]  # pair elem 1 = yb's rem bundle slot
    wpack[:, _WOFF["wr2m"] : _WOFF["wr2m"] + 256] = blk.reshape(128, 256)

    C["wpack1"] = _f8(wpack[:, 0:WSPLIT])
    C["wpack2"] = _f8(wpack[:, WSPLIT:WPACK_COLS])

    # conv2 fold constants: for y2 of parity v, the odd slots in its window
    # contribute -sum(W2*d1): even y2 -> kys {1,3}; odd y2 -> kys {0,2,4}.
    cf_e = ((w2main[:, 1, :] + w2main[:, 3, :]) * d1f[:, None]).sum(0)
    cf_o = ((w2main[:, 0, :] + w2main[:, 2, :] + w2main[:, 4, :]) * d1f[:, None]).sum(0)

    # fc1, permuted to on-chip act2 layout [128, 50, 128] (50 full K-tiles):
    # pair-group p (ya=2p, yb=2p+1) owns blocks 5p..5p+4:
    #   5p+0: ya feats 0:128 ({0,1} DVE) | 5p+1: yb feats 0:128
    #   5p+2: ya feats 128:256           | 5p+3: yb feats 128:256
    #   5p+4: [ya feats 256:320 | yb feats 256:320] (+-1 scalar)
    # {0,1} rows get w'' = 2*d*w (d = sign(s2) flip); fold c3 into fc1 bias.
    w3s = _sign(inp["fc1_w"])  # [120, 6400]
    d2f = np.where(sc2f >= 0, np.float32(1.0), np.float32(-1.0))

    def cols(y2, m):
        return (m // 20) * 400 + y2 * 20 + (m % 20)

    W3T = np.zeros((128, 50, 128), np.float32)
    for p in range(10):
        ya, yb = 2 * p, 2 * p + 1
        m0 = np.arange(128)
        m1 = np.arange(128) + 128
        mm2 = np.arange(64) + 256
        W3T[:, 5 * p + 0, 0:120] = (2 * d2f[m0])[:, None] * w3s[:, cols(ya, m0)].T
        W3T[:, 5 * p + 1, 0:120] = (2 * d2f[m0])[:, None] * w3s[:, cols(yb, m0)].T
        W3T[:, 5 * p + 2, 0:120] = (2 * d2f[m1])[:, None] * w3s[:, cols(ya, m1)].T
        W3T[:, 5 * p + 3, 0:120] = (2 * d2f[m1])[:, None] * w3s[:, cols(yb, m1)].T
        W3T[0:64, 5 * p + 4, 0:120] = w3s[:, cols(ya, mm2)].T
        W3T[64:128, 5 * p + 4, 0:120] = w3s[:, cols(yb, mm2)].T
    C["w3t"] = _f8(W3T)
    # fold constant: c3[m] = sum over {0,1}-encoded inputs of w3s*d
    mdv = np.arange(256)
    c3 = np.zeros(120, np.float32)
    for y2 in range(20):
        c3 += (w3s[:, cols(y2, mdv)] * d2f[mdv][None, :]).sum(1)

    w45 = np.zeros((120, 94), np.float32)
    w45[0:120, 0:84] = _sign(inp["fc2_w"]).T
    w45[0:84, 84:94] = _sign(inp["fc3_w"]).T
    C["w45"] = _bf(w45)

    scp = np.zeros((128, 20), np.float32)
    scp[:, 0], scp[:, 1] = sc1f[:128], bi1f[:128]
    for k in range(8):  # remainder scale/bias at 16-stride
        scp[16 * k : 16 * k + 16, 2] = sc1f[128:]
        scp[16 * k : 16 * k + 16, 3] = bi1f[128:]
    scp[:, 4] = -bi1f[0:128] / sc1f[0:128]        # conv1 odd thr (DVE)
    scp[:, 5] = -bi2f[0:128] / sc2f[0:128] + cf_e[0:128]      # Mt0 thr, even y2
    scp[:, 6] = -bi2f[0:128] / sc2f[0:128] + cf_o[0:128]      # Mt0 thr, odd y2
    scp[:, 7] = -bi2f[128:256] / sc2f[128:256] + cf_e[128:256]
    scp[:, 8] = -bi2f[128:256] / sc2f[128:256] + cf_o[128:256]
    scp[0:64, 9], scp[64:128, 9] = sc2f[256:320], sc2f[256:320]
    scp[0:64, 10] = bi2f[256:320] - sc2f[256:320] * cf_e[256:320]
    scp[64:128, 10] = bi2f[256:320] - sc2f[256:320] * cf_o[256:320]
    scp[0:120, 11], scp[0:120, 12] = s3, b3 - s3 * c3
    scp[0:84, 13], scp[0:84, 14] = s4, b4
    scp[0:10, 15], scp[0:10, 16] = s5, b5
    C["scp"] = _f32c(scp)
    return C


def prep_x(x):
    """sign + feature-major layout + 4 phase shifts: [B,1,28,28] ->
    per-core [4, 128, 7, B_CORE] fp8 (xT row 32y+x = sign(img[y,x]), x<28)."""
    xs = np.sign(x.reshape(B_TOTAL, 28, 28)).astype(np.float32)
    res = []
    for i in range(N_CORES):
        xc = xs[i * B_CORE : (i + 1) * B_CORE]  # [b, 28, 28]
        tmp = np.zeros((B_CORE, 28, 32), np.float32)
        tmp[:, :, 0:28] = xc
        xT = np.zeros((1024, B_CORE), np.float32)
        xT[0:896] = tmp.reshape(B_CORE, 896).T
        xq = np.stack([xT[32 * q : 32 * q + 896].reshape(7, 128, B_CORE).transpose(1, 0, 2)
                       for q in range(4)])  # [4,128,7,b]
        res.append(_f8(xq))
    return res


def build_nc(consts, b_core=B_CORE, chunk=CHUNK, stage=99):
    n_chunks = b_core // chunk
    assert chunk % 128 == 0
    nc = bacc.Bacc(None, target_bir_lowering=False, debug=False)
    xt_in = nc.declare_dram_parameter("xt", [4, 128, 7, b_core], FP8, isOutput=False)
    if stage >= 37:
        out = nc.declare_dram_parameter("out", [10, b_core], F32, isOutput=True)
    else:
        dbg = nc.declare_dram_parameter("dbg", [128, 512], F32, isOutput=True)
    dr = {k: nc.inline_tensor(v, name=f"c_{k}") for k, v in consts.items()}

    with tile.TileContext(nc) as tc, ExitStack() as ctx:
        cp = ctx.enter_context(tc.tile_pool(name="consts", bufs=1))
        xtpool = ctx.enter_context(tc.tile_pool(name="xtpool", bufs=2))
        tpp = ctx.enter_context(tc.tile_pool(name="tp", bufs=2, space="PSUM"))
        cps = ctx.enter_context(tc.tile_pool(name="cps", bufs=2, space="PSUM"))
        fcp = ctx.enter_context(tc.tile_pool(name="fcp", bufs=1, space="PSUM"))
        apool = ctx.enter_context(tc.tile_pool(name="apool", bufs=2))
        a2pool = ctx.enter_context(tc.tile_pool(name="a2pool", bufs=2))
        fpool = ctx.enter_context(tc.tile_pool(name="fpool", bufs=2))
        dpool = ctx.enter_context(tc.tile_pool(name="dpool", bufs=2))

        def load_x(c):
            xtq = [xtpool.tile([128, 7, chunk], FP8, tag=f"xt{q}", name=f"xt{q}")
                   for q in range(4)]
            for q in range(4):
                nc.sync.dma_start(out=xtq[q][:], in_=xt_in[q, :, :, c * chunk : (c + 1) * chunk])
            return xtq

        def cload(name, shape, dtype=FP8):
            t = cp.tile(shape, dtype, tag=f"c_{name}", name=f"c_{name}")
            nc.sync.dma_start(out=t[:], in_=dr[name][:])
            return t

        # issue order: first input slots -> conv1 weights/scales -> the rest
        xtq_next = [xtpool.tile([128, 7, chunk], FP8, tag=f"xt{q}", name=f"xt{q}")
                    for q in range(4)]
        for q in range(4):
            nc.sync.dma_start(out=xtq_next[q][:, 0:3, :], in_=xt_in[q, :, 0:3, 0:chunk])
        wp = cp.tile([128, WPACK_COLS], FP8, tag="c_wpack", name="c_wpack")
        nc.sync.dma_start(out=wp[:, 0:WSPLIT], in_=dr["wpack1"][:])
        scp = cload("scp", [128, 20], F32)
        for q in range(4):
            nc.sync.dma_start(out=xtq_next[q][:, 3:7, :], in_=xt_in[q, :, 3:7, 0:chunk])
        nc.sync.dma_start(out=wp[:, WSPLIT:WPACK_COLS], in_=dr["wpack2"][:])
        w3t = cload("w3t", [128, 50, 128])
        w45 = cload("w45", [120, 94], BF16)

        # HAM warm-up burst: dep-free matmuls fill the input-DMA shadow so the
        # PE clock reaches 2.4 GHz before conv1's first real matmul. vector
        # memset, not gpsimd (gpsimd's first op pays a ~6us IRAM load that
        # would delay the whole burst).
        wub = cp.tile([128, 128], BF16, tag="warm")
        nc.vector.memset(wub[:], 1.0)
        f1w = fcp.tile([128, CHUNK], F32, tag="c1r")
        for _ in range(N_WARM):
            nc.tensor.matmul(f1w[:, 0:128], wub[:], wub[:], start=True, stop=True)

        def wdr(name, Mt=None, g=None, j=None):  # DoubleRow pair view [128, 2, 128]
            o = _WOFF[name]
            if Mt is not None:
                o += 256 * Mt
            if g is not None:
                o += 256 * g
            if j is not None:
                o += 256 * j
            return wp[:, o : o + 256].rearrange("p (a m) -> p a m", a=2)

        def scb(col, p):  # (scale, bias) column pair from scp
            return scp[0:p, col : col + 1], scp[0:p, col + 1 : col + 2]

        # fc2/fc3 of chunk c are software-pipelined into chunk c+1's conv1 so
        # the a3->fc2->a4->fc3 serial act chain hides under real matmuls.
        pend = None  # a3 tile of the previous chunk

        def emit_fc2(a3):
            f2ps = tpp.tile([84, chunk], F32, tag="tp")
            nc.tensor.matmul(f2ps[:], w45[0:120, 0:84], a3[:], start=True, stop=True)
            return f2ps

        def emit_a4(f2ps):
            a4 = fpool.tile([84, chunk], BF16, tag="a4")
            s4_, b4_ = scb(13, 84)
            nc.scalar.activation(a4[:], f2ps[:], AF.Sign, bias=b4_, scale=s4_)
            return a4

        def emit_fc3(a4, cc):
            f3ps = tpp.tile([10, chunk], F32, tag="tp")
            nc.tensor.matmul(f3ps[:], w45[0:84, 84:94], a4[:], start=True, stop=True)
            o5 = fpool.tile([10, chunk], F32, tag="o5")
            s5_, b5_ = scb(15, 10)
            nc.scalar.activation(o5[:], f3ps[:], AF.Identity, bias=b5_, scale=s5_)
            nc.sync.dma_start(out=out[:, cc * chunk : (cc + 1) * chunk], in_=o5[:])

        for c in range(n_chunks):
            xtq = xtq_next
            if stage <= 1:
                dt_ = dpool.tile([128, 512], F32, tag="dbg")
                nc.vector.tensor_copy(out=dt_[:], in_=xtq[1][:, 0, 0:512])
                nc.sync.dma_start(out=dbg[:], in_=dt_[:])
                continue

            # ---- conv1: 1 DoubleRow matmul per (y1, main); remainders of 8 y1
            # grouped block-diagonally into one PSUM bank (1 DR + 1 plain pass
            # per group), output at 16-feature stride = bundle layout.
            # actc slots: 0..23 main y rows (even: scalar +-1, odd: DVE {0,1});
            # 24+3p+s = remainder bundles (+-1).
            actc = apool.tile([128, 48, chunk], FP8, tag="actc")
            f2p_t = a4_t = None
            for yga in range(0, 12, 2):
                ps0s = []
                for yg in (yga, yga + 1):  # 4 DR mains grouped
                    ps0 = cps.tile([128, 2, chunk], F32, tag="cps")
                    ps0s.append(ps0)
                    for ty in range(2):
                        y1 = 2 * yg + ty
                        q, t = y1 % 4, y1 // 4
                        nc.tensor.matmul(ps0[:, ty, :], wdr("w1p0"), xtq[q][:, t : t + 2, :],
                                         start=True, stop=True, perf_mode=DR)
                if yga <= 4:  # remainder group g: 2 passes
                    g = yga // 2
                    c1r = fcp.tile([128, chunk], F32, tag="c1r")
                    nc.tensor.matmul(c1r[:], wdr("w1rA", g=g), xtq[0][:, 2 * g : 2 * g + 2, :],
                                     start=True, stop=False, perf_mode=DR)
                    nc.tensor.matmul(c1r[:], wp[:, _WOFF["w1rB"] + 128 * g : _WOFF["w1rB"] + 128 * g + 128],
                                     xtq[0][:, 2 * g + 2, :], start=False, stop=True)
                if pend is not None and yga == 0:
                    f2p_t = emit_fc2(pend)
                if pend is not None and yga == 4:
                    emit_fc3(a4_t, c - 1)
                    pend = None
                s0, b0 = scb(0, 128)
                thr1 = scp[0:128, 4:5]
                nc.scalar.activation(actc[:, 2 * yga, :], ps0s[0][:, 0, :], AF.Sign,
                                     bias=b0, scale=s0)
                nc.vector.tensor_scalar(actc[:, 2 * yga + 1, :], ps0s[0][:, 1, :],
                                        thr1, None, GE)
                nc.scalar.activation(actc[:, 2 * yga + 2, :], ps0s[1][:, 0, :], AF.Sign,
                                     bias=b0, scale=s0)
                nc.vector.tensor_scalar(actc[:, 2 * yga + 3, :], ps0s[1][:, 1, :],
                                        thr1, None, GE)
                if yga <= 4:
                    s1_, b1_ = scb(2, 128)
                    nc.scalar.activation(actc[:, 24 + yga // 2, :], c1r[:], AF.Sign,
                                         bias=b1_, scale=s1_)
                if f2p_t is not None and yga == 0:
                    a4_t = emit_a4(f2p_t)
                    f2p_t = None
            # 7 phase-shifted copies of the remainder bundles (16-row shifts)
            for p in range(1, 8):
                ns = 3 if p <= 3 else 2
                if p <= 3:
                    nc.vector.memset(actc[:, 24 + 3 * p + 2, :], 0.0)
                nc.sync.dma_start(out=actc[0 : 128 - 16 * p, 24 + 3 * p : 24 + 3 * p + ns, :],
                                  in_=actc[16 * p : 128, 24 : 24 + ns, :])
                nc.sync.dma_start(out=actc[128 - 16 * p : 128, 24 + 3 * p : 24 + 3 * p + 2, :],
                                  in_=actc[0 : 16 * p, 25 : 27, :])
            if c + 1 < n_chunks:  # issue next chunk's input loads early
                xtq_next = load_x(c + 1)
            if stage <= 2:
                dt_ = dpool.tile([128, 512], F32, tag="dbg")
                nc.vector.tensor_copy(out=dt_[:], in_=actc[:, 0, 0:512])
                nc.sync.dma_start(out=dbg[:], in_=dt_[:])
                continue

            # ---- conv2 ----
            # act2 [128, 50, chunk]: pair-group p owns slots 5p..5p+4.
            # Mt0/Mt1 grouped by y2 parity (same threshold column + weight
            # variant for both halves of a PSUM tile): one DVE is_ge per tile
            # writes act2 slots (10pp+2Mt+par, +5) at stride 5.
            act2 = a2pool.tile([128, 50, chunk], FP8, tag="act2")
            for Mt in range(2):
                for pp in range(5):
                    for par in (0, 1):
                        sfx = "o" if par else "e"
                        ps = cps.tile([128, 2, chunk], F32, tag="cps", name=f"c2ps{Mt}")
                        for ty, y2 in ((0, 4 * pp + par), (1, 4 * pp + par + 2)):
                            p8, s8 = y2 % 8, y2 // 8
                            d = (24 + 3 * p8 + s8) - (y2 + 4)
                            nc.tensor.matmul(ps[:, ty, :], wdr("w201" + sfx, Mt=Mt),
                                             actc[:, y2 : y2 + 2, :],
                                             start=True, stop=False, perf_mode=DR)
                            nc.tensor.matmul(ps[:, ty, :], wdr("w223" + sfx, Mt=Mt),
                                             actc[:, y2 + 2 : y2 + 4, :],
                                             start=False, stop=False, perf_mode=DR)
                            nc.tensor.matmul(ps[:, ty, :], wdr("w24r" + sfx, Mt=Mt),
                                             actc[:, y2 + 4 : y2 + 5 + d : d, :],
                                             start=False, stop=True, perf_mode=DR)
                        slot = 10 * pp + 2 * Mt + par
                        nc.vector.tensor_scalar(act2[:, slot : slot + 6 : 5, :], ps[:],
                                                scp[0:128, 5 + 2 * Mt + par : 6 + 2 * Mt + par],
                                                None, GE)
            for p in range(10):  # Mt2 merged: 4 DR passes per pair; scalar act
                ya = 2 * p
                ps = tpp.tile([128, chunk], F32, tag="tp", name="m2ps")
                for j in range(3):
                    nc.tensor.matmul(ps[:], wdr("wm2", j=j),
                                     actc[:, ya + 2 * j : ya + 2 * j + 2, :],
                                     start=(j == 0), stop=False, perf_mode=DR)
                sa = 24 + 3 * (ya % 8) + ya // 8
                nc.tensor.matmul(ps[:], wdr("wr2m"), actc[:, sa : sa + 4 : 3, :],
                                 start=False, stop=True, perf_mode=DR)
                s2_, b2_ = scb(9, 128)
                nc.scalar.activation(act2[:, 5 * p + 4, :], ps[:], AF.Sign,
                                     bias=b2_, scale=s2_)
            if stage <= 3:
                dt_ = dpool.tile([128, 512], F32, tag="dbg")
                nc.vector.tensor_copy(out=dt_[:], in_=act2[:, 0, 0:512])
                nc.sync.dma_start(out=dbg[:], in_=dt_[:])
                continue

            # ---- fc1: 25 DR passes over 50 full K-tiles ----
            f1ps = fcp.tile([128, chunk], F32, tag="f1ps")
            k = 0
            for p in range(10):
                for off in (0, 2):
                    b = 5 * p + off
                    nc.tensor.matmul(f1ps[:], w3t[:, b : b + 2, :], act2[:, b : b + 2, :],
                                     start=(k == 0), stop=False, perf_mode=DR)
                    k += 1
            for q in range(5):
                b = 10 * q + 4
                nc.tensor.matmul(f1ps[:], w3t[:, b : b + 6 : 5, :], act2[:, b : b + 6 : 5, :],
                                 start=False, stop=(q == 4), perf_mode=DR)
            a3 = fpool.tile([120, chunk], BF16, tag="a3")
            s3_, b3_ = scb(11, 120)
            nc.scalar.activation(a3[:], f1ps[0:120, :], AF.Sign, bias=b3_, scale=s3_)
            if stage <= 35:
                dt_ = dpool.tile([128, 512], F32, tag="dbg")
                nc.any.memset(dt_[:], 0.0)
                nc.vector.tensor_copy(out=dt_[0:120, :], in_=a3[:, 0:512])
                nc.sync.dma_start(out=dbg[:], in_=dt_[:])
                continue
            pend = a3  # fc2/fc3 pipelined into the next chunk (or the epilogue)

        if stage >= 37 and pend is not None:
            emit_fc3(emit_a4(emit_fc2(pend)), n_chunks - 1)

    nc.compile()
    return nc


def kernel(**inputs):
    inputs = {k: np.asarray(v) for k, v in inputs.items()}
    consts = build_consts(inputs)
    nc = build_nc(consts)
    xs = prep_x(inputs["x"].astype(np.float32))
    in_maps = [{"xt": xs[i]} for i in range(N_CORES)]
    res = run_bass_kernel_spmd(nc, in_maps, core_ids=list(range(N_CORES)))
    out = np.concatenate([np.asarray(r["out"]).astype(np.float32).T for r in res.results], axis=0)
    return out.astype(np.float32)


# revision 20
# speedup vs baseline: 1.2324x; 1.0542x over previous
"""Binarized LeNet5+BN forward on 8 Trainium2 NeuronCores.

Strategy (data-parallel over batch, 1024 images/core), v2:
  * Feature-major on-chip layout; every layer = matmul-accumulate into PSUM
    followed by ONE activation op (fused conv-bias+BN+hardtanh+binarize).
  * All conv/fc1 matmul operands fp8e4 with perf_mode=DoubleRow (2 K-tiles
    per N=512 pass); host-built Toeplitz +-1/0 weights.
  * v2 pass-count cuts vs v1 (260 -> 217 passes/chunk):
    - conv1 16-feature remainders grouped: 8 y1-rows' remainders packed
      block-diagonally into ONE PSUM bank covering a 384-input-row window
      (1 DR + 1 plain pass per group of 8, x3 groups) instead of 24 passes.
    - conv2 Mt=2 (64-feature) output tiles merged across adjacent y2 pairs
      into full 128-wide passes (4 passes/pair vs 6).
    - fc1 contracts 50 full 128-row act2 tiles = 25 DR passes (vs 30).
  * Activation work split across BOTH ScalarE and VectorE (v1: scalar-only
    at 67% busy was near-critical):
    - Scalar (AF.Sign, +-1 fp8): conv1 mains+remainders, conv2-Mt2-merged,
      fc1, fc2 outputs.
    - Vector (tensor_scalar is_ge, {0,1} fp8): conv2 Mt0/Mt1 outputs.
      Consumers fold the {0,1} encoding: fc1 weight rows for those features
      are 2*w*sign(s2), and the -sum(w*d) constant folds into fc1's Sign
      bias (b3' = b3 - s3*c3). Exact in fp8.
  * Weights packed into few DMA loads issued after the first input tiles;
    HAM warm-up burst before conv1; double-buffered pools throughout.
"""

from contextlib import ExitStack

import ml_dtypes
import numpy as np

import concourse.bacc as bacc
import concourse.tile as tile
from concourse import mybir
from concourse.bass_utils import run_bass_kernel_spmd

F32 = mybir.dt.float32
BF16 = mybir.dt.bfloat16
FP8 = mybir.dt.float8e4
DR = mybir.MatmulPerfMode.DoubleRow
AF = mybir.ActivationFunctionType
GE = mybir.AluOpType.is_ge
EPS = np.float32(1e-5)
N_CORES = 8
B_TOTAL = 8192
B_CORE = B_TOTAL // N_CORES
CHUNK = 512
N_CHUNKS = B_CORE // CHUNK

_f8 = lambda a: np.ascontiguousarray(a.astype(ml_dtypes.float8_e4m3fn))
_bf = lambda a: np.ascontiguousarray(a.astype(ml_dtypes.bfloat16))
_f32c = lambda a: np.ascontiguousarray(a.astype(np.float32))

# wpack column offsets (fp8 [128, 5504]); split for early conv1 load.
# conv2 Mt0/Mt1 weights come in even/odd-y2 variants: odd actc slots are
# DVE-produced {0,1} so their rows carry 2*d1 scaling (see build_consts).
_WOFF = {"w1p0": 0, "w1rA": 256, "w1rB": 1024,
         "w201e": 1408, "w201o": 1920, "w223e": 2432, "w223o": 2944,
         "w24re": 3456, "w24ro": 3968, "wm2": 4480, "wr2m": 5248}
WPACK_COLS = 5504
WSPLIT = 1408  # conv1 weights end
N_WARM = 64  # HAM warm-up matmul count


def _sign(a):
    return np.sign(a).astype(np.float32)


def _toeplitz1(w1s):  # [6,1,5,5] -> [160,144] rows (ky, xi<32), cols (c1,xo)
    W = np.zeros((160, 144), np.float32)
    xo = np.arange(24)
    for ky in range(5):
        for kx in range(5):
            for c1 in range(6):
                W[ky * 32 + xo + kx, c1 * 24 + xo] = w1s[c1, 0, ky, kx]
    return W


def _toeplitz2(w2s):
    """[16,6,5,5] -> main [128,5,320] rows (c1,xi24 mod 128), cols (c2,xo);
    remainder (last 16 rows of each 144-block) at 16-stride: [128,320]."""
    W = np.zeros((720, 320), np.float32)
    xo = np.arange(20)
    for ky in range(5):
        for c1 in range(6):
            for kx in range(5):
                for c2 in range(16):
                    W[ky * 144 + c1 * 24 + xo + kx, c2 * 20 + xo] = w2s[c2, c1, ky, kx]
    main = np.stack([W[144 * k : 144 * k + 128] for k in range(5)], 1)  # [128,5,320]
    rem16 = np.zeros((128, 320), np.float32)  # rows 16k+r (k<5)
    for k in range(5):
        rem16[16 * k : 16 * k + 16] = W[144 * k + 128 : 144 * k + 144]
    return main, rem16


def _affine(g, b, m, v, extra_bias):
    inv = (g.astype(np.float32) / np.sqrt(v.astype(np.float32) + EPS)).astype(np.float32)
    return inv, (inv * (extra_bias.astype(np.float32) - m.astype(np.float32)) + b.astype(np.float32)).astype(np.float32)


def _pair(a, b):  # [128, M] + [<=128, M] -> [128, 2M] interleaved pair-major
    out = np.zeros((128, 2, a.shape[1]), np.float32)
    out[:, 0, :] = a
    out[0 : b.shape[0], 1, :] = b
    return out.reshape(128, -1)


def build_consts(inp):
    """Host-side preprocessing of all weights/BN params into device constants."""
    C = {}
    W1 = _toeplitz1(_sign(inp["conv1_w"]))
    w2main, w2r16 = _toeplitz2(_sign(inp["conv2_w"]))
    wpack = np.zeros((128, WPACK_COLS), np.float32)

    wpack[:, 0:256] = _pair(W1[0:128, 0:128], W1[128:160, 0:128])
    # conv1 remainder groups: group g covers y1 in [8g, 8g+8); its windows
    # span input rows [256g, 256g+384). Col 16k+j <-> (y1=8g+k, feat 128+j).
    for g in range(3):
        A = np.zeros((256, 128), np.float32)
        Bm = np.zeros((128, 128), np.float32)
        for k in range(8):
            full = np.zeros((384, 16), np.float32)
            full[32 * k : 32 * k + 160, :] = W1[:, 128:144]
            A[:, 16 * k : 16 * k + 16] = full[0:256]
            Bm[:, 16 * k : 16 * k + 16] = full[256:384]
        wpack[:, _WOFF["w1rA"] + 256 * g : _WOFF["w1rA"] + 256 * g + 256] = \
            _pair(A[0:128], A[128:256])
        wpack[:, _WOFF["w1rB"] + 128 * g : _WOFF["w1rB"] + 128 * g + 128] = Bm
    # BN affine folds (needed before conv2 packing for the d1 row scaling)
    s1, b1 = _affine(inp["bn1_g"], inp["bn1_b"], inp["bn1_m"], inp["bn1_v"], inp["conv1_b"])
    s2, b2 = _affine(inp["bn2_g"], inp["bn2_b"], inp["bn2_m"], inp["bn2_v"], inp["conv2_b"])
    s3, b3 = _affine(inp["bnf1_g"], inp["bnf1_b"], inp["bnf1_m"], inp["bnf1_v"], inp["fc1_b"])
    s4, b4 = _affine(inp["bnf2_g"], inp["bnf2_b"], inp["bnf2_m"], inp["bnf2_v"], inp["fc2_b"])
    s5, b5 = _affine(inp["bnf3_g"], inp["bnf3_b"], inp["bnf3_m"], inp["bnf3_v"], inp["fc3_b"])
    c1v = np.arange(144) // 24
    sc1f, bi1f = s1[c1v], b1[c1v]
    c2v = np.arange(320) // 20
    sc2f, bi2f = s2[c2v], b2[c2v]
    # conv1 main slots: even y1 -> ScalarE Sign (+-1); odd y1 -> VectorE is_ge
    # ({0,1}); d1 = flip for negative BN scale on the {0,1} decode.
    d1f = np.where(sc1f[0:128] >= 0, np.float32(1.0), np.float32(-1.0))

    def _rsc(w, par_odd):  # scale rows by 2*d1 when the slot parity is odd
        return (2.0 * d1f)[:, None] * w if par_odd else w

    # conv2 Mt0/Mt1: 3 DR pairs each (ky01, ky23, ky4+rem), e/o y2 variants.
    # Pair elem a of pass j touches slot y2+2j+a -> parity (y2+a) % 2.
    for Mt in range(2):
        ms = slice(128 * Mt, 128 * Mt + 128)
        for v, sfx in ((0, "e"), (1, "o")):
            wpack[:, _WOFF["w201" + sfx] + 256 * Mt : _WOFF["w201" + sfx] + 256 * Mt + 256] = \
                _pair(_rsc(w2main[:, 0, ms], (v + 0) % 2), _rsc(w2main[:, 1, ms], (v + 1) % 2))
            wpack[:, _WOFF["w223" + sfx] + 256 * Mt : _WOFF["w223" + sfx] + 256 * Mt + 256] = \
                _pair(_rsc(w2main[:, 2, ms], (v + 0) % 2), _rsc(w2main[:, 3, ms], (v + 1) % 2))
            wpack[:, _WOFF["w24r" + sfx] + 256 * Mt : _WOFF["w24r" + sfx] + 256 * Mt + 256] = \
                _pair(_rsc(w2main[:, 4, ms], (v + 0) % 2), w2r16[:, ms])
    # conv2 Mt2 merged across adjacent (ya, yb=ya+1): cols 0:64 <- ya feats
    # 256:320, cols 64:128 <- yb. Main pass j contracts slots (ya+2j, ya+2j+1);
    # elem a parity = a (ya even).
    m2 = slice(256, 320)
    for j in range(3):
        blk = np.zeros((128, 2, 128), np.float32)
        for a in range(2):
            so = 2 * j + a  # slot offset rel. ya
            if so <= 4:
                blk[:, a, 0:64] = _rsc(w2main[:, so, m2], a % 2)
            if 0 <= so - 1 <= 4:
                blk[:, a, 64:128] = _rsc(w2main[:, so - 1, m2], a % 2)
        wpack[:, _WOFF["wm2"] + 256 * j : _WOFF["wm2"] + 256 * j + 256] = \
            blk.reshape(128, 256)
    blk = np.zeros((128, 2, 128), np.float32)
    blk[:, 0, 0:64] = w2r16[:, m2]   # pair elem 0 = ya's rem bundle slot
    blk[:, 1, 64:128] = w2r16[:, m2]  # pair elem 1 = yb's rem bundle slot
    wpack[:, _WOFF["wr2m"] : _WOFF["wr2m"] + 256] = blk.reshape(128, 256)

    C["wpack1"] = _f8(wpack[:, 0:WSPLIT])
    C["wpack2"] = _f8(wpack[:, WSPLIT:WPACK_COLS])

    # conv2 fold constants: for y2 of parity v, the odd slots in its window
    # contribute -sum(W2*d1): even y2 -> kys {1,3}; odd y2 -> kys {0,2,4}.
    cf_e = ((w2main[:, 1, :] + w2main[:, 3, :]) * d1f[:, None]).sum(0)
    cf_o = ((w2main[:, 0, :] + w2main[:, 2, :] + w2main[:, 4, :]) * d1f[:, None]).sum(0)

    # fc1, permuted to on-chip act2 layout [128, 50, 128] (50 full K-tiles):
    # pair-group p (ya=2p, yb=2p+1) owns blocks 5p..5p+4:
    #   5p+0: ya feats 0:128 ({0,1} DVE) | 5p+1: yb feats 0:128
    #   5p+2: ya feats 128:256           | 5p+3: yb feats 128:256
    #   5p+4: [ya feats 256:320 | yb feats 256:320] (+-1 scalar)
    # {0,1} rows get w'' = 2*d*w (d = sign(s2) flip); fold c3 into fc1 bias.
    w3s = _sign(inp["fc1_w"])  # [120, 6400]
    d2f = np.where(sc2f >= 0, np.float32(1.0), np.float32(-1.0))

    def cols(y2, m):
        return (m // 20) * 400 + y2 * 20 + (m % 20)

    W3T = np.zeros((128, 50, 128), np.float32)
    for p in range(10):
        ya, yb = 2 * p, 2 * p + 1
        m0 = np.arange(128)
        m1 = np.arange(128) + 128
        mm2 = np.arange(64) + 256
        W3T[:, 5 * p + 0, 0:120] = (2 * d2f[m0])[:, None] * w3s[:, cols(ya, m0)].T
        W3T[:, 5 * p + 1, 0:120] = (2 * d2f[m0])[:, None] * w3s[:, cols(yb, m0)].T
        W3T[:, 5 * p + 2, 0:120] = (2 * d2f[m1])[:, None] * w3s[:, cols(ya, m1)].T
        W3T[:, 5 * p + 3, 0:120] = (2 * d2f[m1])[:, None] * w3s[:, cols(yb, m1)].T
        W3T[0:64, 5 * p + 4, 0:120] = w3s[:, cols(ya, mm2)].T
        W3T[64:128, 5 * p + 4, 0:120] = w3s[:, cols(yb, mm2)].T
    C["w3t"] = _f8(W3T)
    # fold constant: c3[m] = sum over {0,1}-encoded inputs of w3s*d
    mdv = np.arange(256)
    c3 = np.zeros(120, np.float32)
    for y2 in range(20):
        c3 += (w3s[:, cols(y2, mdv)] * d2f[mdv][None, :]).sum(1)

    w45 = np.zeros((120, 94), np.float32)
    w45[0:120, 0:84] = _sign(inp["fc2_w"]).T
    w45[0:84, 84:94] = _sign(inp["fc3_w"]).T
    C["w45"] = _bf(w45)

    scp = np.zeros((128, 20), np.float32)
    scp[:, 0], scp[:, 1] = sc1f[:128], bi1f[:128]
    for k in range(8):  # remainder scale/bias at 16-stride
        scp[16 * k : 16 * k + 16, 2] = sc1f[128:]
        scp[16 * k : 16 * k + 16, 3] = bi1f[128:]
    scp[:, 4] = -bi1f[0:128] / sc1f[0:128]        # conv1 odd thr (DVE)
    scp[:, 5] = -bi2f[0:128] / sc2f[0:128] + cf_e[0:128]      # Mt0 thr, even y2
    scp[:, 6] = -bi2f[0:128] / sc2f[0:128] + cf_o[0:128]      # Mt0 thr, odd y2
    scp[:, 7] = -bi2f[128:256] / sc2f[128:256] + cf_e[128:256]
    scp[:, 8] = -bi2f[128:256] / sc2f[128:256] + cf_o[128:256]
    scp[0:64, 9], scp[64:128, 9] = sc2f[256:320], sc2f[256:320]
    scp[0:64, 10] = bi2f[256:320] - sc2f[256:320] * cf_e[256:320]
    scp[64:128, 10] = bi2f[256:320] - sc2f[256:320] * cf_o[256:320]
    scp[0:120, 11], scp[0:120, 12] = s3, b3 - s3 * c3
    scp[0:84, 13], scp[0:84, 14] = s4, b4
    scp[0:10, 15], scp[0:10, 16] = s5, b5
    C["scp"] = _f32c(scp)
    return C


def prep_x(x):
    """sign + feature-major layout + 4 phase shifts: [B,1,28,28] ->
    per-core [4, 128, 7, B_CORE] fp8 (xT row 32y+x = sign(img[y,x]), x<28)."""
    xs = np.sign(x.reshape(B_TOTAL, 28, 28)).astype(np.float32)
    res = []
    for i in range(N_CORES):
        xc = xs[i * B_CORE : (i + 1) * B_CORE]  # [b, 28, 28]
        tmp = np.zeros((B_CORE, 28, 32), np.float32)
        tmp[:, :, 0:28] = xc
        xT = np.zeros((1024, B_CORE), np.float32)
        xT[0:896] = tmp.reshape(B_CORE, 896).T
        xq = np.stack([xT[32 * q : 32 * q + 896].reshape(7, 128, B_CORE).transpose(1, 0, 2)
                       for q in range(4)])  # [4,128,7,b]
        res.append(_f8(xq))
    return res


def build_nc(consts, b_core=B_CORE, chunk=CHUNK, stage=99):
    n_chunks = b_core // chunk
    assert chunk % 128 == 0
    nc = bacc.Bacc(None, target_bir_lowering=False, debug=False)
    xt_in = nc.declare_dram_parameter("xt", [4, 128, 7, b_core], FP8, isOutput=False)
    if stage >= 37:
        out = nc.declare_dram_parameter("out", [10, b_core], F32, isOutput=True)
    else:
        dbg = nc.declare_dram_parameter("dbg", [128, 512], F32, isOutput=True)
    dr = {k: nc.inline_tensor(v, name=f"c_{k}") for k, v in consts.items()}

    with tile.TileContext(nc) as tc, ExitStack() as ctx:
        cp = ctx.enter_context(tc.tile_pool(name="consts", bufs=1))
        xtpool = ctx.enter_context(tc.tile_pool(name="xtpool", bufs=2))
        # PSUM: cps 3 bufs x 2 banks (conv1/conv2 pair tiles, depth-2 act
        # pipelining) + psm 2 bufs x 1 bank (every [*,512] f32 single-bank
        # tile: warmup, conv1 rem, conv2-Mt2m, fc1/fc2/fc3) = 8 banks.
        cps = ctx.enter_context(tc.tile_pool(name="cps", bufs=3, space="PSUM"))
        psm = ctx.enter_context(tc.tile_pool(name="psm", bufs=2, space="PSUM"))
        apool = ctx.enter_context(tc.tile_pool(name="apool", bufs=2))
        a2pool = ctx.enter_context(tc.tile_pool(name="a2pool", bufs=2))
        fpool = ctx.enter_context(tc.tile_pool(name="fpool", bufs=2))
        dpool = ctx.enter_context(tc.tile_pool(name="dpool", bufs=2))

        def load_x(c):
            xtq = [xtpool.tile([128, 7, chunk], FP8, tag=f"xt{q}", name=f"xt{q}")
                   for q in range(4)]
            for q in range(4):
                nc.sync.dma_start(out=xtq[q][:], in_=xt_in[q, :, :, c * chunk : (c + 1) * chunk])
            return xtq

        def cload(name, shape, dtype=FP8):
            t = cp.tile(shape, dtype, tag=f"c_{name}", name=f"c_{name}")
            nc.sync.dma_start(out=t[:], in_=dr[name][:])
            return t

        # issue order: first input slots -> conv1 weights/scales -> the rest
        xtq_next = [xtpool.tile([128, 7, chunk], FP8, tag=f"xt{q}", name=f"xt{q}")
                    for q in range(4)]
        for q in range(4):
            nc.sync.dma_start(out=xtq_next[q][:, 0:3, :], in_=xt_in[q, :, 0:3, 0:chunk])
        wp = cp.tile([128, WPACK_COLS], FP8, tag="c_wpack", name="c_wpack")
        nc.sync.dma_start(out=wp[:, 0:WSPLIT], in_=dr["wpack1"][:])
        scp = cload("scp", [128, 20], F32)
        for q in range(4):
            nc.sync.dma_start(out=xtq_next[q][:, 3:7, :], in_=xt_in[q, :, 3:7, 0:chunk])
        nc.sync.dma_start(out=wp[:, WSPLIT:WPACK_COLS], in_=dr["wpack2"][:])
        w3t = cload("w3t", [128, 50, 128])
        w45 = cload("w45", [120, 94], BF16)

        # HAM warm-up burst: dep-free matmuls fill the input-DMA shadow so the
        # PE clock reaches 2.4 GHz before conv1's first real matmul. vector
        # memset, not gpsimd (gpsimd's first op pays a ~6us IRAM load that
        # would delay the whole burst). Dummy activations pull the one-time
        # ACT_TABLE_LOAD (~1.3us) into the warm-up shadow.
        wub = cp.tile([128, 128], BF16, tag="warm")
        nc.vector.memset(wub[:], 1.0)
        dmt = cp.tile([128, 2], BF16, tag="dmt")
        nc.scalar.activation(dmt[:, 0:1], wub[:, 0:1], AF.Sign)
        nc.scalar.activation(dmt[:, 1:2], wub[:, 0:1], AF.Identity)
        f1w = psm.tile([128, CHUNK], F32, tag="sm")
        for _ in range(N_WARM):
            nc.tensor.matmul(f1w[:, 0:128], wub[:], wub[:], start=True, stop=True)

        def wdr(name, Mt=None, g=None, j=None):  # DoubleRow pair view [128, 2, 128]
            o = _WOFF[name]
            if Mt is not None:
                o += 256 * Mt
            if g is not None:
                o += 256 * g
            if j is not None:
                o += 256 * j
            return wp[:, o : o + 256].rearrange("p (a m) -> p a m", a=2)

        def scb(col, p):  # (scale, bias) column pair from scp
            return scp[0:p, col : col + 1], scp[0:p, col + 1 : col + 2]

        # fc2/fc3 of chunk c are software-pipelined into chunk c+1's conv1 so
        # the a3->fc2->a4->fc3 serial act chain hides under real matmuls.
        pend = None  # a3 tile of the previous chunk

        def emit_fc2(a3):
            f2ps = psm.tile([84, chunk], F32, tag="sm")
            nc.tensor.matmul(f2ps[:], w45[0:120, 0:84], a3[:], start=True, stop=True)
            return f2ps

        def emit_a4(f2ps):
            a4 = fpool.tile([84, chunk], BF16, tag="a4")
            s4_, b4_ = scb(13, 84)
            nc.scalar.activation(a4[:], f2ps[:], AF.Sign, bias=b4_, scale=s4_)
            return a4

        def emit_fc3(a4, cc):
            f3ps = psm.tile([10, chunk], F32, tag="sm")
            nc.tensor.matmul(f3ps[:], w45[0:84, 84:94], a4[:], start=True, stop=True)
            o5 = fpool.tile([10, chunk], F32, tag="o5")
            s5_, b5_ = scb(15, 10)
            nc.scalar.activation(o5[:], f3ps[:], AF.Identity, bias=b5_, scale=s5_)
            nc.sync.dma_start(out=out[:, cc * chunk : (cc + 1) * chunk], in_=o5[:])

        for c in range(n_chunks):
            xtq = xtq_next
            if stage <= 1:
                dt_ = dpool.tile([128, 512], F32, tag="dbg")
                nc.vector.tensor_copy(out=dt_[:], in_=xtq[1][:, 0, 0:512])
                nc.sync.dma_start(out=dbg[:], in_=dt_[:])
                continue

            # ---- conv1: 1 DoubleRow matmul per (y1, main); remainders of 8 y1
            # grouped block-diagonally into one PSUM bank (1 DR + 1 plain pass
            # per group), output at 16-feature stride = bundle layout.
            # actc slots: 0..23 main y rows (even: scalar +-1, odd: DVE {0,1});
            # 24+3p+s = remainder bundles (+-1). Each yga group packs its two
            # even y1 in one PSUM pair-tile and its two odd y1 in the other, so
            # each engine runs ONE [128,2,512] act per group (strided output).
            actc = apool.tile([128, 48, chunk], FP8, tag="actc")
            f2p_t = a4_t = None
            for yga in range(0, 12, 2):
                ps0s = []
                for par in (0, 1):  # evens tile, odds tile
                    ps0 = cps.tile([128, 2, chunk], F32, tag="cps")
                    ps0s.append(ps0)
                    for ty in range(2):
                        y1 = 2 * yga + par + 2 * ty
                        q, t = y1 % 4, y1 // 4
                        nc.tensor.matmul(ps0[:, ty, :], wdr("w1p0"), xtq[q][:, t : t + 2, :],
                                         start=True, stop=True, perf_mode=DR)
                if 4 <= yga <= 8:  # remainder group g: 2 passes
                    g = yga // 2 - 2
                    c1r = psm.tile([128, chunk], F32, tag="sm")
                    nc.tensor.matmul(c1r[:], wdr("w1rA", g=g), xtq[0][:, 2 * g : 2 * g + 2, :],
                                     start=True, stop=False, perf_mode=DR)
                    nc.tensor.matmul(c1r[:], wp[:, _WOFF["w1rB"] + 128 * g : _WOFF["w1rB"] + 128 * g + 128],
                                     xtq[0][:, 2 * g + 2, :], start=False, stop=True)
                if pend is not None and yga == 0:
                    f2p_t = emit_fc2(pend)
                if pend is not None and yga == 6:
                    emit_fc3(a4_t, c - 1)
                    pend = None
                s0, b0 = scb(0, 128)
                nc.scalar.activation(actc[:, 2 * yga : 2 * yga + 3 : 2, :], ps0s[0][:],
                                     AF.Sign, bias=b0, scale=s0)
                nc.vector.tensor_scalar(actc[:, 2 * yga + 1 : 2 * yga + 4 : 2, :],
                                        ps0s[1][:], scp[0:128, 4:5], None, GE)
                if 4 <= yga <= 8:
                    s1_, b1_ = scb(2, 128)
                    nc.scalar.activation(actc[:, 24 + yga // 2 - 2, :], c1r[:], AF.Sign,
                                         bias=b1_, scale=s1_)
                if f2p_t is not None and yga == 2:
                    a4_t = emit_a4(f2p_t)
                    f2p_t = None
            # 7 phase-shifted copies of the remainder bundles (16-row shifts)
            for p in range(1, 8):
                ns = 3 if p <= 3 else 2
                if p <= 3:
                    nc.vector.memset(actc[:, 24 + 3 * p + 2, :], 0.0)
                nc.sync.dma_start(out=actc[0 : 128 - 16 * p, 24 + 3 * p : 24 + 3 * p + ns, :],
                                  in_=actc[16 * p : 128, 24 : 24 + ns, :])
                nc.sync.dma_start(out=actc[128 - 16 * p : 128, 24 + 3 * p : 24 + 3 * p + 2, :],
                                  in_=actc[0 : 16 * p, 25 : 27, :])
            if c + 1 < n_chunks:  # issue next chunk's input loads early
                xtq_next = load_x(c + 1)
            if stage <= 2:
                dt_ = dpool.tile([128, 512], F32, tag="dbg")
                nc.vector.tensor_copy(out=dt_[:], in_=actc[:, 0, 0:512])
                nc.sync.dma_start(out=dbg[:], in_=dt_[:])
                continue

            # ---- conv2 ----
            # act2 [128, 50, chunk]: pair-group p owns slots 5p..5p+4.
            # Mt0/Mt1 grouped by y2 parity (same threshold column + weight
            # variant for both halves of a PSUM tile): one DVE is_ge per tile
            # writes act2 slots (10pp+2Mt+par, +5) at stride 5.
            act2 = a2pool.tile([128, 50, chunk], FP8, tag="act2")
            for Mt in range(2):
                for pp in range(5):
                    for par in (0, 1):
                        sfx = "o" if par else "e"
                        ps = cps.tile([128, 2, chunk], F32, tag="cps", name=f"c2ps{Mt}")
                        for ty, y2 in ((0, 4 * pp + par), (1, 4 * pp + par + 2)):
                            p8, s8 = y2 % 8, y2 // 8
                            d = (24 + 3 * p8 + s8) - (y2 + 4)
                            nc.tensor.matmul(ps[:, ty, :], wdr("w201" + sfx, Mt=Mt),
                                             actc[:, y2 : y2 + 2, :],
                                             start=True, stop=False, perf_mode=DR)
                            nc.tensor.matmul(ps[:, ty, :], wdr("w223" + sfx, Mt=Mt),
                                             actc[:, y2 + 2 : y2 + 4, :],
                                             start=False, stop=False, perf_mode=DR)
                            nc.tensor.matmul(ps[:, ty, :], wdr("w24r" + sfx, Mt=Mt),
                                             actc[:, y2 + 4 : y2 + 5 + d : d, :],
                                             start=False, stop=True, perf_mode=DR)
                        slot = 10 * pp + 2 * Mt + par
                        nc.vector.tensor_scalar(act2[:, slot : slot + 6 : 5, :], ps[:],
                                                scp[0:128, 5 + 2 * Mt + par : 6 + 2 * Mt + par],
                                                None, GE)
            for p in range(10):  # Mt2 merged: 4 DR passes per pair; scalar act
                ya = 2 * p
                ps = psm.tile([128, chunk], F32, tag="sm", name="m2ps")
                for j in range(3):
                    nc.tensor.matmul(ps[:], wdr("wm2", j=j),
                                     actc[:, ya + 2 * j : ya + 2 * j + 2, :],
                                     start=(j == 0), stop=False, perf_mode=DR)
                sa = 24 + 3 * (ya % 8) + ya // 8
                nc.tensor.matmul(ps[:], wdr("wr2m"), actc[:, sa : sa + 4 : 3, :],
                                 start=False, stop=True, perf_mode=DR)
                s2_, b2_ = scb(9, 128)
                nc.scalar.activation(act2[:, 5 * p + 4, :], ps[:], AF.Sign,
                                     bias=b2_, scale=s2_)
            if stage <= 3:
                dt_ = dpool.tile([128, 512], F32, tag="dbg")
                nc.vector.tensor_copy(out=dt_[:], in_=act2[:, 0, 0:512])
                nc.sync.dma_start(out=dbg[:], in_=dt_[:])
                continue

            # ---- fc1: 25 DR passes over 50 full K-tiles ----
            f1ps = psm.tile([128, chunk], F32, tag="sm", name="f1ps")
            k = 0
            for p in range(10):
                for off in (0, 2):
                    b = 5 * p + off
                    nc.tensor.matmul(f1ps[:], w3t[:, b : b + 2, :], act2[:, b : b + 2, :],
                                     start=(k == 0), stop=False, perf_mode=DR)
                    k += 1
            for q in range(5):
                b = 10 * q + 4
                nc.tensor.matmul(f1ps[:], w3t[:, b : b + 6 : 5, :], act2[:, b : b + 6 : 5, :],
                                 start=False, stop=(q == 4), perf_mode=DR)
            a3 = fpool.tile([120, chunk], BF16, tag="a3")
            s3_, b3_ = scb(11, 120)
            nc.scalar.activation(a3[:], f1ps[0:120, :], AF.Sign, bias=b3_, scale=s3_)
            if stage <= 35:
                dt_ = dpool.tile([128, 512], F32, tag="dbg")
                nc.any.memset(dt_[:], 0.0)
                nc.vector.tensor_copy(out=dt_[0:120, :], in_=a3[:, 0:512])
                nc.sync.dma_start(out=dbg[:], in_=dt_[:])
                continue
            pend = a3  # fc2/fc3 pipelined into the next chunk (or the epilogue)

        if stage >= 37 and pend is not None:
            emit_fc3(emit_a4(emit_fc2(pend)), n_chunks - 1)

    nc.compile()
    return nc


def kernel(**inputs):
    inputs = {k: np.asarray(v) for k, v in inputs.items()}
    consts = build_consts(inputs)
    nc = build_nc(consts)
    xs = prep_x(inputs["x"].astype(np.float32))
    in_maps = [{"xt": xs[i]} for i in range(N_CORES)]
    res = run_bass_kernel_spmd(nc, in_maps, core_ids=list(range(N_CORES)))
    out = np.concatenate([np.asarray(r["out"]).astype(np.float32).T for r in res.results], axis=0)
    return out.astype(np.float32)


# revision 24
# speedup vs baseline: 1.2339x; 1.0012x over previous
"""Binarized LeNet5+BN forward on 8 Trainium2 NeuronCores.

Strategy (data-parallel over batch, 1024 images/core), v2:
  * Feature-major on-chip layout; every layer = matmul-accumulate into PSUM
    followed by ONE activation op (fused conv-bias+BN+hardtanh+binarize).
  * All conv/fc1 matmul operands fp8e4 with perf_mode=DoubleRow (2 K-tiles
    per N=512 pass); host-built Toeplitz +-1/0 weights.
  * v2 pass-count cuts vs v1 (260 -> 217 passes/chunk):
    - conv1 16-feature remainders grouped: 8 y1-rows' remainders packed
      block-diagonally into ONE PSUM bank covering a 384-input-row window
      (1 DR + 1 plain pass per group of 8, x3 groups) instead of 24 passes.
    - conv2 Mt=2 (64-feature) output tiles merged across adjacent y2 pairs
      into full 128-wide passes (4 passes/pair vs 6).
    - fc1 contracts 50 full 128-row act2 tiles = 25 DR passes (vs 30).
  * Activation work split across BOTH ScalarE and VectorE (v1: scalar-only
    at 67% busy was near-critical):
    - Scalar (AF.Sign, +-1 fp8): conv1 mains+remainders, conv2-Mt2-merged,
      fc1, fc2 outputs.
    - Vector (tensor_scalar is_ge, {0,1} fp8): conv2 Mt0/Mt1 outputs.
      Consumers fold the {0,1} encoding: fc1 weight rows for those features
      are 2*w*sign(s2), and the -sum(w*d) constant folds into fc1's Sign
      bias (b3' = b3 - s3*c3). Exact in fp8.
  * Weights packed into few DMA loads issued after the first input tiles;
    HAM warm-up burst before conv1; double-buffered pools throughout.
"""

from contextlib import ExitStack

import ml_dtypes
import numpy as np

import concourse.bacc as bacc
import concourse.tile as tile
from concourse import mybir
from concourse.bass_utils import run_bass_kernel_spmd

F32 = mybir.dt.float32
BF16 = mybir.dt.bfloat16
FP8 = mybir.dt.float8e4
DR = mybir.MatmulPerfMode.DoubleRow
AF = mybir.ActivationFunctionType
GE = mybir.AluOpType.is_ge
EPS = np.float32(1e-5)
N_CORES = 8
B_TOTAL = 8192
B_CORE = B_TOTAL // N_CORES
CHUNK = 512
N_CHUNKS = B_CORE // CHUNK

_f8 = lambda a: np.ascontiguousarray(a.astype(ml_dtypes.float8_e4m3fn))
_bf = lambda a: np.ascontiguousarray(a.astype(ml_dtypes.bfloat16))
_f32c = lambda a: np.ascontiguousarray(a.astype(np.float32))

# wpack column offsets (fp8 [128, 5504]); split for early conv1 load.
# conv2 Mt0/Mt1 weights come in even/odd-y2 variants: odd actc slots are
# DVE-produced {0,1} so their rows carry 2*d1 scaling (see build_consts).
_WOFF = {"w1p0": 0, "w1rA": 256, "w1rB": 1024,
         "w201e": 1408, "w201o": 1920, "w223e": 2432, "w223o": 2944,
         "w24re": 3456, "w24ro": 3968, "wm2": 4480, "wr2m": 5248}
WPACK_COLS = 5504
WSPLIT = 1408  # conv1 weights end
N_WARM = 64  # HAM warm-up matmul count


def _sign(a):
    return np.sign(a).astype(np.float32)


def _toeplitz1(w1s):  # [6,1,5,5] -> [160,144] rows (ky, xi<32), cols (c1,xo)
    W = np.zeros((160, 144), np.float32)
    xo = np.arange(24)
    for ky in range(5):
        for kx in range(5):
            for c1 in range(6):
                W[ky * 32 + xo + kx, c1 * 24 + xo] = w1s[c1, 0, ky, kx]
    return W


def _toeplitz2(w2s):
    """[16,6,5,5] -> main [128,5,320] rows (c1,xi24 mod 128), cols (c2,xo);
    remainder (last 16 rows of each 144-block) at 16-stride: [128,320]."""
    W = np.zeros((720, 320), np.float32)
    xo = np.arange(20)
    for ky in range(5):
        for c1 in range(6):
            for kx in range(5):
                for c2 in range(16):
                    W[ky * 144 + c1 * 24 + xo + kx, c2 * 20 + xo] = w2s[c2, c1, ky, kx]
    main = np.stack([W[144 * k : 144 * k + 128] for k in range(5)], 1)  # [128,5,320]
    rem16 = np.zeros((128, 320), np.float32)  # rows 16k+r (k<5)
    for k in range(5):
        rem16[16 * k : 16 * k + 16] = W[144 * k + 128 : 144 * k + 144]
    return main, rem16


def _affine(g, b, m, v, extra_bias):
    inv = (g.astype(np.float32) / np.sqrt(v.astype(np.float32) + EPS)).astype(np.float32)
    return inv, (inv * (extra_bias.astype(np.float32) - m.astype(np.float32)) + b.astype(np.float32)).astype(np.float32)


def _pair(a, b):  # [128, M] + [<=128, M] -> [128, 2M] interleaved pair-major
    out = np.zeros((128, 2, a.shape[1]), np.float32)
    out[:, 0, :] = a
    out[0 : b.shape[0], 1, :] = b
    return out.reshape(128, -1)


def build_consts(inp):
    """Host-side preprocessing of all weights/BN params into device constants."""
    C = {}
    W1 = _toeplitz1(_sign(inp["conv1_w"]))
    w2main, w2r16 = _toeplitz2(_sign(inp["conv2_w"]))
    wpack = np.zeros((128, WPACK_COLS), np.float32)

    wpack[:, 0:256] = _pair(W1[0:128, 0:128], W1[128:160, 0:128])
    # conv1 remainder groups: group g covers y1 in [8g, 8g+8); its windows
    # span input rows [256g, 256g+384). Col 16k+j <-> (y1=8g+k, feat 128+j).
    for g in range(3):
        A = np.zeros((256, 128), np.float32)
        Bm = np.zeros((128, 128), np.float32)
        for k in range(8):
            full = np.zeros((384, 16), np.float32)
            full[32 * k : 32 * k + 160, :] = W1[:, 128:144]
            A[:, 16 * k : 16 * k + 16] = full[0:256]
            Bm[:, 16 * k : 16 * k + 16] = full[256:384]
        wpack[:, _WOFF["w1rA"] + 256 * g : _WOFF["w1rA"] + 256 * g + 256] = \
            _pair(A[0:128], A[128:256])
        wpack[:, _WOFF["w1rB"] + 128 * g : _WOFF["w1rB"] + 128 * g + 128] = Bm
    # BN affine folds (needed before conv2 packing for the d1 row scaling)
    s1, b1 = _affine(inp["bn1_g"], inp["bn1_b"], inp["bn1_m"], inp["bn1_v"], inp["conv1_b"])
    s2, b2 = _affine(inp["bn2_g"], inp["bn2_b"], inp["bn2_m"], inp["bn2_v"], inp["conv2_b"])
    s3, b3 = _affine(inp["bnf1_g"], inp["bnf1_b"], inp["bnf1_m"], inp["bnf1_v"], inp["fc1_b"])
    s4, b4 = _affine(inp["bnf2_g"], inp["bnf2_b"], inp["bnf2_m"], inp["bnf2_v"], inp["fc2_b"])
    s5, b5 = _affine(inp["bnf3_g"], inp["bnf3_b"], inp["bnf3_m"], inp["bnf3_v"], inp["fc3_b"])
    c1v = np.arange(144) // 24
    sc1f, bi1f = s1[c1v], b1[c1v]
    c2v = np.arange(320) // 20
    sc2f, bi2f = s2[c2v], b2[c2v]
    # conv1 main slots: even y1 -> ScalarE Sign (+-1); odd y1 -> VectorE is_ge
    # ({0,1}); d1 = flip for negative BN scale on the {0,1} decode.
    d1f = np.where(sc1f[0:128] >= 0, np.float32(1.0), np.float32(-1.0))

    def _rsc(w, par_odd):  # scale rows by 2*d1 when the slot parity is odd
        return (2.0 * d1f)[:, None] * w if par_odd else w

    # conv2 Mt0/Mt1: 3 DR pairs each (ky01, ky23, ky4+rem), e/o y2 variants.
    # Pair elem a of pass j touches slot y2+2j+a -> parity (y2+a) % 2.
    for Mt in range(2):
        ms = slice(128 * Mt, 128 * Mt + 128)
        for v, sfx in ((0, "e"), (1, "o")):
            wpack[:, _WOFF["w201" + sfx] + 256 * Mt : _WOFF["w201" + sfx] + 256 * Mt + 256] = \
                _pair(_rsc(w2main[:, 0, ms], (v + 0) % 2), _rsc(w2main[:, 1, ms], (v + 1) % 2))
            wpack[:, _WOFF["w223" + sfx] + 256 * Mt : _WOFF["w223" + sfx] + 256 * Mt + 256] = \
                _pair(_rsc(w2main[:, 2, ms], (v + 0) % 2), _rsc(w2main[:, 3, ms], (v + 1) % 2))
            wpack[:, _WOFF["w24r" + sfx] + 256 * Mt : _WOFF["w24r" + sfx] + 256 * Mt + 256] = \
                _pair(_rsc(w2main[:, 4, ms], (v + 0) % 2), w2r16[:, ms])
    # conv2 Mt2 merged across adjacent (ya, yb=ya+1): cols 0:64 <- ya feats
    # 256:320, cols 64:128 <- yb. Main pass j contracts slots (ya+2j, ya+2j+1);
    # elem a parity = a (ya even).
    m2 = slice(256, 320)
    for j in range(3):
        blk = np.zeros((128, 2, 128), np.float32)
        for a in range(2):
            so = 2 * j + a  # slot offset rel. ya
            if so <= 4:
                blk[:, a, 0:64] = _rsc(w2main[:, so, m2], a % 2)
            if 0 <= so - 1 <= 4:
                blk[:, a, 64:128] = _rsc(w2main[:, so - 1, m2], a % 2)
        wpack[:, _WOFF["wm2"] + 256 * j : _WOFF["wm2"] + 256 * j + 256] = \
            blk.reshape(128, 256)
    blk = np.zeros((128, 2, 128), np.float32)
    blk[:, 0, 0:64] = w2r16[:, m2]   # pair elem 0 = ya's rem bundle slot
    blk[:, 1, 64:128] = w2r16[:, m2]  # pair elem 1 = yb's rem bundle slot
    wpack[:, _WOFF["wr2m"] : _WOFF["wr2m"] + 256] = blk.reshape(128, 256)

    C["wpack1"] = _f8(wpack[:, 0:WSPLIT])
    C["wpack2"] = _f8(wpack[:, WSPLIT:WPACK_COLS])

    # conv2 fold constants: for y2 of parity v, the odd slots in its window
    # contribute -sum(W2*d1): even y2 -> kys {1,3}; odd y2 -> kys {0,2,4}.
    cf_e = ((w2main[:, 1, :] + w2main[:, 3, :]) * d1f[:, None]).sum(0)
    cf_o = ((w2main[:, 0, :] + w2main[:, 2, :] + w2main[:, 4, :]) * d1f[:, None]).sum(0)

    # fc1, permuted to on-chip act2 layout [128, 50, 128] (50 full K-tiles):
    # pair-group p (ya=2p, yb=2p+1) owns blocks 5p..5p+4:
    #   5p+0: ya feats 0:128 ({0,1} DVE) | 5p+1: yb feats 0:128
    #   5p+2: ya feats 128:256           | 5p+3: yb feats 128:256
    #   5p+4: [ya feats 256:320 | yb feats 256:320] (+-1 scalar)
    # {0,1} rows get w'' = 2*d*w (d = sign(s2) flip); fold c3 into fc1 bias.
    w3s = _sign(inp["fc1_w"])  # [120, 6400]
    d2f = np.where(sc2f >= 0, np.float32(1.0), np.float32(-1.0))

    def cols(y2, m):
        return (m // 20) * 400 + y2 * 20 + (m % 20)

    W3T = np.zeros((128, 50, 128), np.float32)
    for p in range(10):
        ya, yb = 2 * p, 2 * p + 1
        m0 = np.arange(128)
        m1 = np.arange(128) + 128
        mm2 = np.arange(64) + 256
        W3T[:, 5 * p + 0, 0:120] = (2 * d2f[m0])[:, None] * w3s[:, cols(ya, m0)].T
        W3T[:, 5 * p + 1, 0:120] = (2 * d2f[m0])[:, None] * w3s[:, cols(yb, m0)].T
        W3T[:, 5 * p + 2, 0:120] = (2 * d2f[m1])[:, None] * w3s[:, cols(ya, m1)].T
        W3T[:, 5 * p + 3, 0:120] = (2 * d2f[m1])[:, None] * w3s[:, cols(yb, m1)].T
        W3T[0:64, 5 * p + 4, 0:120] = w3s[:, cols(ya, mm2)].T
        W3T[64:128, 5 * p + 4, 0:120] = w3s[:, cols(yb, mm2)].T
    C["w3t"] = _f8(W3T)
    # fold constant: c3[m] = sum over {0,1}-encoded inputs of w3s*d
    mdv = np.arange(256)
    c3 = np.zeros(120, np.float32)
    for y2 in range(20):
        c3 += (w3s[:, cols(y2, mdv)] * d2f[mdv][None, :]).sum(1)

    w45 = np.zeros((120, 94), np.float32)
    w45[0:120, 0:84] = _sign(inp["fc2_w"]).T
    w45[0:84, 84:94] = _sign(inp["fc3_w"]).T
    C["w45"] = _bf(w45)

    scp = np.zeros((128, 20), np.float32)
    scp[:, 0], scp[:, 1] = sc1f[:128], bi1f[:128]
    for k in range(8):  # remainder scale/bias at 16-stride
        scp[16 * k : 16 * k + 16, 2] = sc1f[128:]
        scp[16 * k : 16 * k + 16, 3] = bi1f[128:]
    scp[:, 4] = -bi1f[0:128] / sc1f[0:128]        # conv1 odd thr (DVE)
    scp[:, 5] = -bi2f[0:128] / sc2f[0:128] + cf_e[0:128]      # Mt0 thr, even y2
    scp[:, 6] = -bi2f[0:128] / sc2f[0:128] + cf_o[0:128]      # Mt0 thr, odd y2
    scp[:, 7] = -bi2f[128:256] / sc2f[128:256] + cf_e[128:256]
    scp[:, 8] = -bi2f[128:256] / sc2f[128:256] + cf_o[128:256]
    scp[0:64, 9], scp[64:128, 9] = sc2f[256:320], sc2f[256:320]
    scp[0:64, 10] = bi2f[256:320] - sc2f[256:320] * cf_e[256:320]
    scp[64:128, 10] = bi2f[256:320] - sc2f[256:320] * cf_o[256:320]
    scp[0:120, 11], scp[0:120, 12] = s3, b3 - s3 * c3
    scp[0:84, 13], scp[0:84, 14] = s4, b4
    scp[0:10, 15], scp[0:10, 16] = s5, b5
    C["scp"] = _f32c(scp)
    return C


def prep_x(x):
    """sign + feature-major layout + 4 phase shifts: [B,1,28,28] -> per-core
    [N_CHUNKS, 4, 128, 7, CHUNK] fp8 (xT row 32y+x = sign(img[y,x]), x<28).
    Chunk-outer so each on-device load is one fully-contiguous transfer."""
    xs = np.sign(x.reshape(B_TOTAL, 28, 28)).astype(np.float32)
    res = []
    for i in range(N_CORES):
        xc = xs[i * B_CORE : (i + 1) * B_CORE]  # [b, 28, 28]
        tmp = np.zeros((B_CORE, 28, 32), np.float32)
        tmp[:, :, 0:28] = xc
        xT = np.zeros((1024, B_CORE), np.float32)
        xT[0:896] = tmp.reshape(B_CORE, 896).T
        xq = np.stack([xT[32 * q : 32 * q + 896].reshape(7, 128, B_CORE).transpose(1, 0, 2)
                       for q in range(4)])  # [4,128,7,b]
        xqc = xq.reshape(4, 128, 7, N_CHUNKS, CHUNK).transpose(3, 0, 1, 2, 4)
        res.append(_f8(np.ascontiguousarray(xqc)))
    return res


def build_nc(consts, b_core=B_CORE, chunk=CHUNK, stage=99):
    n_chunks = b_core // chunk
    assert chunk % 128 == 0
    nc = bacc.Bacc(None, target_bir_lowering=False, debug=False)
    xt_in = nc.declare_dram_parameter("xt", [n_chunks, 4, 128, 7, chunk], FP8, isOutput=False)
    if stage >= 37:
        out = nc.declare_dram_parameter("out", [10, b_core], F32, isOutput=True)
    else:
        dbg = nc.declare_dram_parameter("dbg", [128, 512], F32, isOutput=True)
    dr = {k: nc.inline_tensor(v, name=f"c_{k}") for k, v in consts.items()}

    with tile.TileContext(nc) as tc, ExitStack() as ctx:
        cp = ctx.enter_context(tc.tile_pool(name="consts", bufs=1))
        xtpool = ctx.enter_context(tc.tile_pool(name="xtpool", bufs=2))
        # PSUM: cps 3 bufs x 2 banks (conv1/conv2 pair tiles, depth-2 act
        # pipelining) + psm 2 bufs x 1 bank (every [*,512] f32 single-bank
        # tile: warmup, conv1 rem, conv2-Mt2m, fc1/fc2/fc3) = 8 banks.
        cps = ctx.enter_context(tc.tile_pool(name="cps", bufs=3, space="PSUM"))
        psm = ctx.enter_context(tc.tile_pool(name="psm", bufs=2, space="PSUM"))
        apool = ctx.enter_context(tc.tile_pool(name="apool", bufs=2))
        a2pool = ctx.enter_context(tc.tile_pool(name="a2pool", bufs=2))
        fpool = ctx.enter_context(tc.tile_pool(name="fpool", bufs=2))
        dpool = ctx.enter_context(tc.tile_pool(name="dpool", bufs=2))

        def load_x(c):
            xtq = [xtpool.tile([128, 7, chunk], FP8, tag=f"xt{q}", name=f"xt{q}")
                   for q in range(4)]
            for q in range(4):
                nc.sync.dma_start(out=xtq[q][:], in_=xt_in[c, q])
            return xtq

        def cload(name, shape, dtype=FP8):
            t = cp.tile(shape, dtype, tag=f"c_{name}", name=f"c_{name}")
            nc.sync.dma_start(out=t[:], in_=dr[name][:])
            return t

        # issue order: first input slots -> conv1 weights/scales -> the rest
        xtq_next = [xtpool.tile([128, 7, chunk], FP8, tag=f"xt{q}", name=f"xt{q}")
                    for q in range(4)]
        for q in range(4):
            nc.sync.dma_start(out=xtq_next[q][:, 0:3, :], in_=xt_in[0, q, :, 0:3, :])
        wp = cp.tile([128, WPACK_COLS], FP8, tag="c_wpack", name="c_wpack")
        nc.sync.dma_start(out=wp[:, 0:WSPLIT], in_=dr["wpack1"][:])
        scp = cload("scp", [128, 20], F32)
        for q in range(4):
            nc.sync.dma_start(out=xtq_next[q][:, 3:7, :], in_=xt_in[0, q, :, 3:7, :])
        nc.sync.dma_start(out=wp[:, WSPLIT:WPACK_COLS], in_=dr["wpack2"][:])
        w3t = cload("w3t", [128, 50, 128])
        w45 = cload("w45", [120, 94], BF16)

        # HAM warm-up burst: dep-free matmuls fill the input-DMA shadow so the
        # PE clock reaches 2.4 GHz before conv1's first real matmul. vector
        # memset, not gpsimd (gpsimd's first op pays a ~6us IRAM load that
        # would delay the whole burst). Dummy activations pull the one-time
        # ACT_TABLE_LOAD (~1.3us) into the warm-up shadow.
        wub = cp.tile([128, 128], BF16, tag="warm")
        nc.vector.memset(wub[:], 1.0)
        dmt = cp.tile([128, 2], BF16, tag="dmt")
        nc.scalar.activation(dmt[:, 0:1], wub[:, 0:1], AF.Sign)
        nc.scalar.activation(dmt[:, 1:2], wub[:, 0:1], AF.Identity)
        f1w = psm.tile([128, CHUNK], F32, tag="sm")
        for _ in range(N_WARM):
            nc.tensor.matmul(f1w[:, 0:128], wub[:], wub[:], start=True, stop=True)

        def wdr(name, Mt=None, g=None, j=None):  # DoubleRow pair view [128, 2, 128]
            o = _WOFF[name]
            if Mt is not None:
                o += 256 * Mt
            if g is not None:
                o += 256 * g
            if j is not None:
                o += 256 * j
            return wp[:, o : o + 256].rearrange("p (a m) -> p a m", a=2)

        def scb(col, p):  # (scale, bias) column pair from scp
            return scp[0:p, col : col + 1], scp[0:p, col + 1 : col + 2]

        # fc2/fc3 of chunk c are software-pipelined into chunk c+1's conv1 so
        # the a3->fc2->a4->fc3 serial act chain hides under real matmuls.
        pend = None  # a3 tile of the previous chunk

        def emit_fc2(a3):
            f2ps = psm.tile([84, chunk], F32, tag="sm")
            nc.tensor.matmul(f2ps[:], w45[0:120, 0:84], a3[:], start=True, stop=True)
            return f2ps

        def emit_a4(f2ps):
            a4 = fpool.tile([84, chunk], BF16, tag="a4")
            s4_, b4_ = scb(13, 84)
            nc.scalar.activation(a4[:], f2ps[:], AF.Sign, bias=b4_, scale=s4_)
            return a4

        def emit_fc3(a4, cc):
            f3ps = psm.tile([10, chunk], F32, tag="sm")
            nc.tensor.matmul(f3ps[:], w45[0:84, 84:94], a4[:], start=True, stop=True)
            o5 = fpool.tile([10, chunk], F32, tag="o5")
            s5_, b5_ = scb(15, 10)
            nc.scalar.activation(o5[:], f3ps[:], AF.Identity, bias=b5_, scale=s5_)
            nc.sync.dma_start(out=out[:, cc * chunk : (cc + 1) * chunk], in_=o5[:])

        for c in range(n_chunks):
            xtq = xtq_next
            if stage <= 1:
                dt_ = dpool.tile([128, 512], F32, tag="dbg")
                nc.vector.tensor_copy(out=dt_[:], in_=xtq[1][:, 0, 0:512])
                nc.sync.dma_start(out=dbg[:], in_=dt_[:])
                continue

            # ---- conv1: 1 DoubleRow matmul per (y1, main); remainders of 8 y1
            # grouped block-diagonally into one PSUM bank (1 DR + 1 plain pass
            # per group), output at 16-feature stride = bundle layout.
            # actc slots: 0..23 main y rows (even: scalar +-1, odd: DVE {0,1});
            # 24+3p+s = remainder bundles (+-1). Each yga group packs its two
            # even y1 in one PSUM pair-tile and its two odd y1 in the other, so
            # each engine runs ONE [128,2,512] act per group (strided output).
            actc = apool.tile([128, 48, chunk], FP8, tag="actc")
            f2p_t = a4_t = None
            for yga in range(0, 12, 2):
                ps0s = []
                for par in (0, 1):  # evens tile, odds tile
                    ps0 = cps.tile([128, 2, chunk], F32, tag="cps")
                    ps0s.append(ps0)
                    for ty in range(2):
                        y1 = 2 * yga + par + 2 * ty
                        q, t = y1 % 4, y1 // 4
                        nc.tensor.matmul(ps0[:, ty, :], wdr("w1p0"), xtq[q][:, t : t + 2, :],
                                         start=True, stop=True, perf_mode=DR)
                if 4 <= yga <= 8:  # remainder group g: 2 passes
                    g = yga // 2 - 2
                    c1r = psm.tile([128, chunk], F32, tag="sm")
                    nc.tensor.matmul(c1r[:], wdr("w1rA", g=g), xtq[0][:, 2 * g : 2 * g + 2, :],
                                     start=True, stop=False, perf_mode=DR)
                    nc.tensor.matmul(c1r[:], wp[:, _WOFF["w1rB"] + 128 * g : _WOFF["w1rB"] + 128 * g + 128],
                                     xtq[0][:, 2 * g + 2, :], start=False, stop=True)
                if pend is not None and yga == 0:
                    f2p_t = emit_fc2(pend)
                if pend is not None and yga == 6:
                    emit_fc3(a4_t, c - 1)
                    pend = None
                s0, b0 = scb(0, 128)
                nc.scalar.activation(actc[:, 2 * yga : 2 * yga + 3 : 2, :], ps0s[0][:],
                                     AF.Sign, bias=b0, scale=s0)
                nc.vector.tensor_scalar(actc[:, 2 * yga + 1 : 2 * yga + 4 : 2, :],
                                        ps0s[1][:], scp[0:128, 4:5], None, GE)
                if 4 <= yga <= 8:
                    s1_, b1_ = scb(2, 128)
                    nc.scalar.activation(actc[:, 24 + yga // 2 - 2, :], c1r[:], AF.Sign,
                                         bias=b1_, scale=s1_)
                if f2p_t is not None and yga == 2:
                    a4_t = emit_a4(f2p_t)
                    f2p_t = None
            # 7 phase-shifted copies of the remainder bundles (16-row shifts)
            for p in range(1, 8):
                ns = 3 if p <= 3 else 2
                if p <= 3:
                    nc.vector.memset(actc[:, 24 + 3 * p + 2, :], 0.0)
                nc.sync.dma_start(out=actc[0 : 128 - 16 * p, 24 + 3 * p : 24 + 3 * p + ns, :],
                                  in_=actc[16 * p : 128, 24 : 24 + ns, :])
                nc.sync.dma_start(out=actc[128 - 16 * p : 128, 24 + 3 * p : 24 + 3 * p + 2, :],
                                  in_=actc[0 : 16 * p, 25 : 27, :])
            if c + 1 < n_chunks:  # issue next chunk's input loads early
                xtq_next = load_x(c + 1)
            if stage <= 2:
                dt_ = dpool.tile([128, 512], F32, tag="dbg")
                nc.vector.tensor_copy(out=dt_[:], in_=actc[:, 0, 0:512])
                nc.sync.dma_start(out=dbg[:], in_=dt_[:])
                continue

            # ---- conv2 ----
            # act2 [128, 50, chunk]: pair-group p owns slots 5p..5p+4.
            # Mt0/Mt1 grouped by y2 parity (same threshold column + weight
            # variant for both halves of a PSUM tile): one DVE is_ge per tile
            # writes act2 slots (10pp+2Mt+par, +5) at stride 5.
            act2 = a2pool.tile([128, 50, chunk], FP8, tag="act2")
            for Mt in range(2):
                for pp in range(5):
                    for par in (0, 1):
                        sfx = "o" if par else "e"
                        ps = cps.tile([128, 2, chunk], F32, tag="cps", name=f"c2ps{Mt}")
                        for ty, y2 in ((0, 4 * pp + par), (1, 4 * pp + par + 2)):
                            p8, s8 = y2 % 8, y2 // 8
                            d = (24 + 3 * p8 + s8) - (y2 + 4)
                            nc.tensor.matmul(ps[:, ty, :], wdr("w201" + sfx, Mt=Mt),
                                             actc[:, y2 : y2 + 2, :],
                                             start=True, stop=False, perf_mode=DR)
                            nc.tensor.matmul(ps[:, ty, :], wdr("w223" + sfx, Mt=Mt),
                                             actc[:, y2 + 2 : y2 + 4, :],
                                             start=False, stop=False, perf_mode=DR)
                            nc.tensor.matmul(ps[:, ty, :], wdr("w24r" + sfx, Mt=Mt),
                                             actc[:, y2 + 4 : y2 + 5 + d : d, :],
                                             start=False, stop=True, perf_mode=DR)
                        slot = 10 * pp + 2 * Mt + par
                        nc.vector.tensor_scalar(act2[:, slot : slot + 6 : 5, :], ps[:],
                                                scp[0:128, 5 + 2 * Mt + par : 6 + 2 * Mt + par],
                                                None, GE)
            for p in range(10):  # Mt2 merged: 4 DR passes per pair; scalar act
                ya = 2 * p
                ps = psm.tile([128, chunk], F32, tag="sm", name="m2ps")
                for j in range(3):
                    nc.tensor.matmul(ps[:], wdr("wm2", j=j),
                                     actc[:, ya + 2 * j : ya + 2 * j + 2, :],
                                     start=(j == 0), stop=False, perf_mode=DR)
                sa = 24 + 3 * (ya % 8) + ya // 8
                nc.tensor.matmul(ps[:], wdr("wr2m"), actc[:, sa : sa + 4 : 3, :],
                                 start=False, stop=True, perf_mode=DR)
                s2_, b2_ = scb(9, 128)
                nc.scalar.activation(act2[:, 5 * p + 4, :], ps[:], AF.Sign,
                                     bias=b2_, scale=s2_)
            if stage <= 3:
                dt_ = dpool.tile([128, 512], F32, tag="dbg")
                nc.vector.tensor_copy(out=dt_[:], in_=act2[:, 0, 0:512])
                nc.sync.dma_start(out=dbg[:], in_=dt_[:])
                continue

            # ---- fc1: 25 DR passes over 50 full K-tiles ----
            f1ps = psm.tile([128, chunk], F32, tag="sm", name="f1ps")
            k = 0
            for p in range(10):
                for off in (0, 2):
                    b = 5 * p + off
                    nc.tensor.matmul(f1ps[:], w3t[:, b : b + 2, :], act2[:, b : b + 2, :],
                                     start=(k == 0), stop=False, perf_mode=DR)
                    k += 1
            for q in range(5):
                b = 10 * q + 4
                nc.tensor.matmul(f1ps[:], w3t[:, b : b + 6 : 5, :], act2[:, b : b + 6 : 5, :],
                                 start=False, stop=(q == 4), perf_mode=DR)
            a3 = fpool.tile([120, chunk], BF16, tag="a3")
            s3_, b3_ = scb(11, 120)
            nc.scalar.activation(a3[:], f1ps[0:120, :], AF.Sign, bias=b3_, scale=s3_)
            if stage <= 35:
                dt_ = dpool.tile([128, 512], F32, tag="dbg")
                nc.any.memset(dt_[:], 0.0)
                nc.vector.tensor_copy(out=dt_[0:120, :], in_=a3[:, 0:512])
                nc.sync.dma_start(out=dbg[:], in_=dt_[:])
                continue
            pend = a3  # fc2/fc3 pipelined into the next chunk (or the epilogue)

        if stage >= 37 and pend is not None:
            emit_fc3(emit_a4(emit_fc2(pend)), n_chunks - 1)

    nc.compile()
    return nc


def kernel(**inputs):
    inputs = {k: np.asarray(v) for k, v in inputs.items()}
    consts = build_consts(inputs)
    nc = build_nc(consts)
    xs = prep_x(inputs["x"].astype(np.float32))
    in_maps = [{"xt": xs[i]} for i in range(N_CORES)]
    res = run_bass_kernel_spmd(nc, in_maps, core_ids=list(range(N_CORES)))
    out = np.concatenate([np.asarray(r["out"]).astype(np.float32).T for r in res.results], axis=0)
    return out.astype(np.float32)


# revision 25
# speedup vs baseline: 1.2685x; 1.0281x over previous
"""Binarized LeNet5+BN forward on 8 Trainium2 NeuronCores.

Strategy (data-parallel over batch, 1024 images/core), v2:
  * Feature-major on-chip layout; every layer = matmul-accumulate into PSUM
    followed by ONE activation op (fused conv-bias+BN+hardtanh+binarize).
  * All conv/fc1 matmul operands fp8e4 with perf_mode=DoubleRow (2 K-tiles
    per N=512 pass); host-built Toeplitz +-1/0 weights.
  * v2 pass-count cuts vs v1 (260 -> 217 passes/chunk):
    - conv1 16-feature remainders grouped: 8 y1-rows' remainders packed
      block-diagonally into ONE PSUM bank covering a 384-input-row window
      (1 DR + 1 plain pass per group of 8, x3 groups) instead of 24 passes.
    - conv2 Mt=2 (64-feature) output tiles merged across adjacent y2 pairs
      into full 128-wide passes (4 passes/pair vs 6).
    - fc1 contracts 50 full 128-row act2 tiles = 25 DR passes (vs 30).
  * Activation work split across BOTH ScalarE and VectorE (v1: scalar-only
    at 67% busy was near-critical):
    - Scalar (AF.Sign, +-1 fp8): conv1 mains+remainders, conv2-Mt2-merged,
      fc1, fc2 outputs.
    - Vector (tensor_scalar is_ge, {0,1} fp8): conv2 Mt0/Mt1 outputs.
      Consumers fold the {0,1} encoding: fc1 weight rows for those features
      are 2*w*sign(s2), and the -sum(w*d) constant folds into fc1's Sign
      bias (b3' = b3 - s3*c3). Exact in fp8.
  * Weights packed into few DMA loads issued after the first input tiles;
    HAM warm-up burst before conv1; double-buffered pools throughout.
"""

from contextlib import ExitStack

import ml_dtypes
import numpy as np

import concourse.bacc as bacc
import concourse.tile as tile
from concourse import mybir
from concourse.bass_utils import run_bass_kernel_spmd

F32 = mybir.dt.float32
BF16 = mybir.dt.bfloat16
FP8 = mybir.dt.float8e4
DR = mybir.MatmulPerfMode.DoubleRow
AF = mybir.ActivationFunctionType
GE = mybir.AluOpType.is_ge
EPS = np.float32(1e-5)
N_CORES = 8
B_TOTAL = 8192
B_CORE = B_TOTAL // N_CORES
CHUNK = 512
N_CHUNKS = B_CORE // CHUNK

_f8 = lambda a: np.ascontiguousarray(a.astype(ml_dtypes.float8_e4m3fn))
_bf = lambda a: np.ascontiguousarray(a.astype(ml_dtypes.bfloat16))
_f32c = lambda a: np.ascontiguousarray(a.astype(np.float32))

# wpack column offsets (fp8 [128, 5504]); split for early conv1 load.
# conv2 Mt0/Mt1 weights come in even/odd-y2 variants: odd actc slots are
# DVE-produced {0,1} so their rows carry 2*d1 scaling (see build_consts).
_WOFF = {"w1p0": 0, "w1rA": 256, "w1rB": 1024,
         "w201e": 1408, "w201o": 1920, "w223e": 2432, "w223o": 2944,
         "w24re": 3456, "w24ro": 3968, "wm2": 4480, "wr2m": 5248}
WPACK_COLS = 5504
WSPLIT = 1408  # conv1 weights end
N_WARM = 64  # HAM warm-up matmul count


def _sign(a):
    return np.sign(a).astype(np.float32)


def _toeplitz1(w1s):  # [6,1,5,5] -> [160,144] rows (ky, xi<32), cols (c1,xo)
    W = np.zeros((160, 144), np.float32)
    xo = np.arange(24)
    for ky in range(5):
        for kx in range(5):
            for c1 in range(6):
                W[ky * 32 + xo + kx, c1 * 24 + xo] = w1s[c1, 0, ky, kx]
    return W


def _toeplitz2(w2s):
    """[16,6,5,5] -> main [128,5,320] rows (c1,xi24 mod 128), cols (c2,xo);
    remainder (last 16 rows of each 144-block) at 16-stride: [128,320]."""
    W = np.zeros((720, 320), np.float32)
    xo = np.arange(20)
    for ky in range(5):
        for c1 in range(6):
            for kx in range(5):
                for c2 in range(16):
                    W[ky * 144 + c1 * 24 + xo + kx, c2 * 20 + xo] = w2s[c2, c1, ky, kx]
    main = np.stack([W[144 * k : 144 * k + 128] for k in range(5)], 1)  # [128,5,320]
    rem16 = np.zeros((128, 320), np.float32)  # rows 16k+r (k<5)
    for k in range(5):
        rem16[16 * k : 16 * k + 16] = W[144 * k + 128 : 144 * k + 144]
    return main, rem16


def _affine(g, b, m, v, extra_bias):
    inv = (g.astype(np.float32) / np.sqrt(v.astype(np.float32) + EPS)).astype(np.float32)
    return inv, (inv * (extra_bias.astype(np.float32) - m.astype(np.float32)) + b.astype(np.float32)).astype(np.float32)


def _pair(a, b):  # [128, M] + [<=128, M] -> [128, 2M] interleaved pair-major
    out = np.zeros((128, 2, a.shape[1]), np.float32)
    out[:, 0, :] = a
    out[0 : b.shape[0], 1, :] = b
    return out.reshape(128, -1)


def build_consts(inp):
    """Host-side preprocessing of all weights/BN params into device constants."""
    C = {}
    W1 = _toeplitz1(_sign(inp["conv1_w"]))
    w2main, w2r16 = _toeplitz2(_sign(inp["conv2_w"]))
    wpack = np.zeros((128, WPACK_COLS), np.float32)

    wpack[:, 0:256] = _pair(W1[0:128, 0:128], W1[128:160, 0:128])
    # conv1 remainder groups: group g covers y1 in [8g, 8g+8); its windows
    # span input rows [256g, 256g+384). Col 16k+j <-> (y1=8g+k, feat 128+j).
    for g in range(3):
        A = np.zeros((256, 128), np.float32)
        Bm = np.zeros((128, 128), np.float32)
        for k in range(8):
            full = np.zeros((384, 16), np.float32)
            full[32 * k : 32 * k + 160, :] = W1[:, 128:144]
            A[:, 16 * k : 16 * k + 16] = full[0:256]
            Bm[:, 16 * k : 16 * k + 16] = full[256:384]
        wpack[:, _WOFF["w1rA"] + 256 * g : _WOFF["w1rA"] + 256 * g + 256] = \
            _pair(A[0:128], A[128:256])
        wpack[:, _WOFF["w1rB"] + 128 * g : _WOFF["w1rB"] + 128 * g + 128] = Bm
    # BN affine folds (needed before conv2 packing for the d1 row scaling)
    s1, b1 = _affine(inp["bn1_g"], inp["bn1_b"], inp["bn1_m"], inp["bn1_v"], inp["conv1_b"])
    s2, b2 = _affine(inp["bn2_g"], inp["bn2_b"], inp["bn2_m"], inp["bn2_v"], inp["conv2_b"])
    s3, b3 = _affine(inp["bnf1_g"], inp["bnf1_b"], inp["bnf1_m"], inp["bnf1_v"], inp["fc1_b"])
    s4, b4 = _affine(inp["bnf2_g"], inp["bnf2_b"], inp["bnf2_m"], inp["bnf2_v"], inp["fc2_b"])
    s5, b5 = _affine(inp["bnf3_g"], inp["bnf3_b"], inp["bnf3_m"], inp["bnf3_v"], inp["fc3_b"])
    c1v = np.arange(144) // 24
    sc1f, bi1f = s1[c1v], b1[c1v]
    c2v = np.arange(320) // 20
    sc2f, bi2f = s2[c2v], b2[c2v]
    # conv1 main slots: even y1 -> ScalarE Sign (+-1); odd y1 -> VectorE is_ge
    # ({0,1}); d1 = flip for negative BN scale on the {0,1} decode.
    d1f = np.where(sc1f[0:128] >= 0, np.float32(1.0), np.float32(-1.0))

    def _rsc(w, par_odd):  # scale rows by 2*d1 when the slot parity is odd
        return (2.0 * d1f)[:, None] * w if par_odd else w

    # conv2 Mt0/Mt1: 3 DR pairs each (ky01, ky23, ky4+rem), e/o y2 variants.
    # Pair elem a of pass j touches slot y2+2j+a -> parity (y2+a) % 2.
    for Mt in range(2):
        ms = slice(128 * Mt, 128 * Mt + 128)
        for v, sfx in ((0, "e"), (1, "o")):
            wpack[:, _WOFF["w201" + sfx] + 256 * Mt : _WOFF["w201" + sfx] + 256 * Mt + 256] = \
                _pair(_rsc(w2main[:, 0, ms], (v + 0) % 2), _rsc(w2main[:, 1, ms], (v + 1) % 2))
            wpack[:, _WOFF["w223" + sfx] + 256 * Mt : _WOFF["w223" + sfx] + 256 * Mt + 256] = \
                _pair(_rsc(w2main[:, 2, ms], (v + 0) % 2), _rsc(w2main[:, 3, ms], (v + 1) % 2))
            wpack[:, _WOFF["w24r" + sfx] + 256 * Mt : _WOFF["w24r" + sfx] + 256 * Mt + 256] = \
                _pair(_rsc(w2main[:, 4, ms], (v + 0) % 2), w2r16[:, ms])
    # conv2 Mt2 merged across adjacent (ya, yb=ya+1): cols 0:64 <- ya feats
    # 256:320, cols 64:128 <- yb. Main pass j contracts slots (ya+2j, ya+2j+1);
    # elem a parity = a (ya even).
    m2 = slice(256, 320)
    for j in range(3):
        blk = np.zeros((128, 2, 128), np.float32)
        for a in range(2):
            so = 2 * j + a  # slot offset rel. ya
            if so <= 4:
                blk[:, a, 0:64] = _rsc(w2main[:, so, m2], a % 2)
            if 0 <= so - 1 <= 4:
                blk[:, a, 64:128] = _rsc(w2main[:, so - 1, m2], a % 2)
        wpack[:, _WOFF["wm2"] + 256 * j : _WOFF["wm2"] + 256 * j + 256] = \
            blk.reshape(128, 256)
    blk = np.zeros((128, 2, 128), np.float32)
    blk[:, 0, 0:64] = w2r16[:, m2]   # pair elem 0 = ya's rem bundle slot
    blk[:, 1, 64:128] = w2r16[:, m2]  # pair elem 1 = yb's rem bundle slot
    wpack[:, _WOFF["wr2m"] : _WOFF["wr2m"] + 256] = blk.reshape(128, 256)

    C["wpack1"] = _f8(wpack[:, 0:WSPLIT])
    C["wpack2"] = _f8(wpack[:, WSPLIT:WPACK_COLS])

    # conv2 fold constants: for y2 of parity v, the odd slots in its window
    # contribute -sum(W2*d1): even y2 -> kys {1,3}; odd y2 -> kys {0,2,4}.
    cf_e = ((w2main[:, 1, :] + w2main[:, 3, :]) * d1f[:, None]).sum(0)
    cf_o = ((w2main[:, 0, :] + w2main[:, 2, :] + w2main[:, 4, :]) * d1f[:, None]).sum(0)

    # fc1, permuted to on-chip act2 layout [128, 50, 128] (50 full K-tiles):
    # pair-group p (ya=2p, yb=2p+1) owns blocks 5p..5p+4:
    #   5p+0: ya feats 0:128 ({0,1} DVE) | 5p+1: yb feats 0:128
    #   5p+2: ya feats 128:256           | 5p+3: yb feats 128:256
    #   5p+4: [ya feats 256:320 | yb feats 256:320] (+-1 scalar)
    # {0,1} rows get w'' = 2*d*w (d = sign(s2) flip); fold c3 into fc1 bias.
    w3s = _sign(inp["fc1_w"])  # [120, 6400]
    d2f = np.where(sc2f >= 0, np.float32(1.0), np.float32(-1.0))

    def cols(y2, m):
        return (m // 20) * 400 + y2 * 20 + (m % 20)

    W3T = np.zeros((128, 50, 128), np.float32)
    for p in range(10):
        ya, yb = 2 * p, 2 * p + 1
        m0 = np.arange(128)
        m1 = np.arange(128) + 128
        mm2 = np.arange(64) + 256
        W3T[:, 5 * p + 0, 0:120] = (2 * d2f[m0])[:, None] * w3s[:, cols(ya, m0)].T
        W3T[:, 5 * p + 1, 0:120] = (2 * d2f[m0])[:, None] * w3s[:, cols(yb, m0)].T
        W3T[:, 5 * p + 2, 0:120] = (2 * d2f[m1])[:, None] * w3s[:, cols(ya, m1)].T
        W3T[:, 5 * p + 3, 0:120] = (2 * d2f[m1])[:, None] * w3s[:, cols(yb, m1)].T
        W3T[0:64, 5 * p + 4, 0:120] = w3s[:, cols(ya, mm2)].T
        W3T[64:128, 5 * p + 4, 0:120] = w3s[:, cols(yb, mm2)].T
    C["w3t"] = _f8(W3T)
    # fold constant: c3[m] = sum over {0,1}-encoded inputs of w3s*d
    mdv = np.arange(256)
    c3 = np.zeros(120, np.float32)
    for y2 in range(20):
        c3 += (w3s[:, cols(y2, mdv)] * d2f[mdv][None, :]).sum(1)

    w45 = np.zeros((120, 94), np.float32)
    w45[0:120, 0:84] = _sign(inp["fc2_w"]).T
    w45[0:84, 84:94] = _sign(inp["fc3_w"]).T
    C["w45"] = _bf(w45)

    scp = np.zeros((128, 20), np.float32)
    scp[:, 0], scp[:, 1] = sc1f[:128], bi1f[:128]
    for k in range(8):  # remainder scale/bias at 16-stride
        scp[16 * k : 16 * k + 16, 2] = sc1f[128:]
        scp[16 * k : 16 * k + 16, 3] = bi1f[128:]
    scp[:, 4] = -bi1f[0:128] / sc1f[0:128]        # conv1 odd thr (DVE)
    scp[:, 5] = -bi2f[0:128] / sc2f[0:128] + cf_e[0:128]      # Mt0 thr, even y2
    scp[:, 6] = -bi2f[0:128] / sc2f[0:128] + cf_o[0:128]      # Mt0 thr, odd y2
    scp[:, 7] = -bi2f[128:256] / sc2f[128:256] + cf_e[128:256]
    scp[:, 8] = -bi2f[128:256] / sc2f[128:256] + cf_o[128:256]
    scp[0:64, 9], scp[64:128, 9] = sc2f[256:320], sc2f[256:320]
    scp[0:64, 10] = bi2f[256:320] - sc2f[256:320] * cf_e[256:320]
    scp[64:128, 10] = bi2f[256:320] - sc2f[256:320] * cf_o[256:320]
    scp[0:120, 11], scp[0:120, 12] = s3, b3 - s3 * c3
    scp[0:84, 13], scp[0:84, 14] = s4, b4
    scp[0:10, 15], scp[0:10, 16] = s5, b5
    C["scp"] = _f32c(scp)
    return C


def prep_x(x):
    """sign + feature-major layout + 4 phase shifts: [B,1,28,28] -> per-core
    [N_CHUNKS, 4, 128, 7, CHUNK] fp8 (xT row 32y+x = sign(img[y,x]), x<28).
    Chunk-outer so each on-device load is one fully-contiguous transfer."""
    xs = np.sign(x.reshape(B_TOTAL, 28, 28)).astype(np.float32)
    res = []
    for i in range(N_CORES):
        xc = xs[i * B_CORE : (i + 1) * B_CORE]  # [b, 28, 28]
        tmp = np.zeros((B_CORE, 28, 32), np.float32)
        tmp[:, :, 0:28] = xc
        xT = np.zeros((1024, B_CORE), np.float32)
        xT[0:896] = tmp.reshape(B_CORE, 896).T
        xq = np.stack([xT[32 * q : 32 * q + 896].reshape(7, 128, B_CORE).transpose(1, 0, 2)
                       for q in range(4)])  # [4,128,7,b]
        xqc = xq.reshape(4, 128, 7, N_CHUNKS, CHUNK).transpose(3, 0, 1, 2, 4)
        res.append(_f8(np.ascontiguousarray(xqc)))
    return res


def build_nc(consts, b_core=B_CORE, chunk=CHUNK, stage=99):
    n_chunks = b_core // chunk
    assert chunk % 128 == 0
    nc = bacc.Bacc(None, target_bir_lowering=False, debug=False)
    xt_in = nc.declare_dram_parameter("xt", [n_chunks, 4, 128, 7, chunk], FP8, isOutput=False)
    if stage >= 37:
        out = nc.declare_dram_parameter("out", [10, b_core], F32, isOutput=True)
    else:
        dbg = nc.declare_dram_parameter("dbg", [128, 512], F32, isOutput=True)
    dr = {k: nc.inline_tensor(v, name=f"c_{k}") for k, v in consts.items()}

    with tile.TileContext(nc) as tc, ExitStack() as ctx:
        cp = ctx.enter_context(tc.tile_pool(name="consts", bufs=1))
        xtpool = ctx.enter_context(tc.tile_pool(name="xtpool", bufs=2))
        # PSUM: cps 3 bufs x 2 banks (conv1/conv2 pair tiles, depth-2 act
        # pipelining) + psm 2 bufs x 1 bank (every [*,512] f32 single-bank
        # tile: warmup, conv1 rem, conv2-Mt2m, fc1/fc2/fc3) = 8 banks.
        cps = ctx.enter_context(tc.tile_pool(name="cps", bufs=3, space="PSUM"))
        psm = ctx.enter_context(tc.tile_pool(name="psm", bufs=2, space="PSUM"))
        apool = ctx.enter_context(tc.tile_pool(name="apool", bufs=2))
        a2pool = ctx.enter_context(tc.tile_pool(name="a2pool", bufs=2))
        fpool = ctx.enter_context(tc.tile_pool(name="fpool", bufs=2))
        dpool = ctx.enter_context(tc.tile_pool(name="dpool", bufs=2))

        def load_x(c):
            xtq = [xtpool.tile([128, 7, chunk], FP8, tag=f"xt{q}", name=f"xt{q}")
                   for q in range(4)]
            for q in range(4):
                nc.sync.dma_start(out=xtq[q][:], in_=xt_in[c, q])
            return xtq

        def cload(name, shape, dtype=FP8):
            t = cp.tile(shape, dtype, tag=f"c_{name}", name=f"c_{name}")
            nc.sync.dma_start(out=t[:], in_=dr[name][:])
            return t

        # issue order: first input slots -> conv1 weights/scales -> the rest
        xtq_next = [xtpool.tile([128, 7, chunk], FP8, tag=f"xt{q}", name=f"xt{q}")
                    for q in range(4)]
        for q in range(4):
            nc.sync.dma_start(out=xtq_next[q][:, 0:3, :], in_=xt_in[0, q, :, 0:3, :])
        wp = cp.tile([128, WPACK_COLS], FP8, tag="c_wpack", name="c_wpack")
        nc.sync.dma_start(out=wp[:, 0:WSPLIT], in_=dr["wpack1"][:])
        scp = cload("scp", [128, 20], F32)
        for q in range(4):
            nc.sync.dma_start(out=xtq_next[q][:, 3:7, :], in_=xt_in[0, q, :, 3:7, :])
        nc.sync.dma_start(out=wp[:, WSPLIT:WPACK_COLS], in_=dr["wpack2"][:])
        w3t = cload("w3t", [128, 50, 128])
        w45 = cload("w45", [120, 94], BF16)

        # HAM warm-up burst: dep-free matmuls fill the input-DMA shadow so the
        # PE clock reaches 2.4 GHz before conv1's first real matmul. vector
        # memset, not gpsimd (gpsimd's first op pays a ~6us IRAM load that
        # would delay the whole burst). Dummy activations pull the one-time
        # ACT_TABLE_LOAD (~1.3us) into the warm-up shadow.
        wub = cp.tile([128, 128], BF16, tag="warm")
        nc.vector.memset(wub[:], 1.0)
        dmt = cp.tile([128, 2], BF16, tag="dmt")
        nc.scalar.activation(dmt[:, 0:1], wub[:, 0:1], AF.Sign)
        nc.scalar.activation(dmt[:, 1:2], wub[:, 0:1], AF.Identity)
        f1w = psm.tile([128, CHUNK], F32, tag="sm")
        for _ in range(N_WARM):
            nc.tensor.matmul(f1w[:, 0:128], wub[:], wub[:], start=True, stop=True)

        def wdr(name, Mt=None, g=None, j=None):  # DoubleRow pair view [128, 2, 128]
            o = _WOFF[name]
            if Mt is not None:
                o += 256 * Mt
            if g is not None:
                o += 256 * g
            if j is not None:
                o += 256 * j
            return wp[:, o : o + 256].rearrange("p (a m) -> p a m", a=2)

        def scb(col, p):  # (scale, bias) column pair from scp
            return scp[0:p, col : col + 1], scp[0:p, col + 1 : col + 2]

        # fc2/fc3 of chunk c are software-pipelined into chunk c+1's conv1 so
        # the a3->fc2->a4->fc3 serial act chain hides under real matmuls.
        pend = None  # a3 tile of the previous chunk

        def emit_fc2(a3):
            f2ps = psm.tile([84, chunk], F32, tag="sm")
            nc.tensor.matmul(f2ps[:], w45[0:120, 0:84], a3[:], start=True, stop=True)
            return f2ps

        def emit_a4(f2ps):
            a4 = fpool.tile([84, chunk], BF16, tag="a4")
            s4_, b4_ = scb(13, 84)
            nc.scalar.activation(a4[:], f2ps[:], AF.Sign, bias=b4_, scale=s4_)
            return a4

        def emit_fc3(a4, cc):
            f3ps = psm.tile([10, chunk], F32, tag="sm")
            nc.tensor.matmul(f3ps[:], w45[0:84, 84:94], a4[:], start=True, stop=True)
            o5 = fpool.tile([10, chunk], F32, tag="o5")
            s5_, b5_ = scb(15, 10)
            nc.scalar.activation(o5[:], f3ps[:], AF.Identity, bias=b5_, scale=s5_)
            nc.sync.dma_start(out=out[:, cc * chunk : (cc + 1) * chunk], in_=o5[:])

        for c in range(n_chunks):
            xtq = xtq_next
            if stage <= 1:
                dt_ = dpool.tile([128, 512], F32, tag="dbg")
                nc.vector.tensor_copy(out=dt_[:], in_=xtq[1][:, 0, 0:512])
                nc.sync.dma_start(out=dbg[:], in_=dt_[:])
                continue

            # ---- conv1: 1 DoubleRow matmul per (y1, main); remainders of 8 y1
            # grouped block-diagonally into one PSUM bank (1 DR + 1 plain pass
            # per group), output at 16-feature stride = bundle layout.
            # actc slots: 0..23 main y rows (even: scalar +-1, odd: DVE {0,1});
            # 24+3p+s = remainder bundles (+-1). Each yga group packs its two
            # even y1 in one PSUM pair-tile and its two odd y1 in the other, so
            # each engine runs ONE [128,2,512] act per group (strided output).
            actc = apool.tile([128, 48, chunk], FP8, tag="actc")
            act2 = a2pool.tile([128, 50, chunk], FP8, tag="act2")

            def c2pair(Mt, ya, yb):
                # conv2 Mt0/Mt1 pair (ya, yb) of equal parity: 6 DR passes +
                # one DVE is_ge writing act2 slots (5*(y//2)+2Mt+par).
                par = ya % 2
                sfx = "o" if par else "e"
                ps = cps.tile([128, 2, chunk], F32, tag="cps", name=f"c2ps{Mt}")
                for ty, y2 in ((0, ya), (1, yb)):
                    p8, s8 = y2 % 8, y2 // 8
                    d = (24 + 3 * p8 + s8) - (y2 + 4)
                    nc.tensor.matmul(ps[:, ty, :], wdr("w201" + sfx, Mt=Mt),
                                     actc[:, y2 : y2 + 2, :],
                                     start=True, stop=False, perf_mode=DR)
                    nc.tensor.matmul(ps[:, ty, :], wdr("w223" + sfx, Mt=Mt),
                                     actc[:, y2 + 2 : y2 + 4, :],
                                     start=False, stop=False, perf_mode=DR)
                    nc.tensor.matmul(ps[:, ty, :], wdr("w24r" + sfx, Mt=Mt),
                                     actc[:, y2 + 4 : y2 + 5 + d : d, :],
                                     start=False, stop=True, perf_mode=DR)
                sa = 5 * (ya // 2) + 2 * Mt + par
                sb = 5 * (yb // 2) + 2 * Mt + par
                nc.vector.tensor_scalar(act2[:, sa : sb + 1 : sb - sa, :], ps[:],
                                        scp[0:128, 5 + 2 * Mt + par : 6 + 2 * Mt + par],
                                        None, GE)

            # conv2 pairs (y, y+8): pair 0 needs no bundle phase-copies, pair k
            # needs copy k -- ordered so the interleave below never waits.
            PAIRS2 = [(0, 8), (1, 9), (2, 10), (3, 11), (4, 12),
                      (5, 13), (6, 14), (7, 15), (16, 18), (17, 19)]
            # conv1 groups with conv2-Mt0 pairs interleaved once enough actc
            # slots exist: the PE rides conv2 passes while conv1's act chain
            # (the latency-bound part) drains in the background.
            ILV = {3: [0], 4: [1, 2, 3], 5: [4, 5, 6, 7, 8, 9]}
            f2p_t = a4_t = None
            for gi, yga in enumerate(range(0, 12, 2)):
                ps0s = []
                for par in (0, 1):  # evens tile, odds tile
                    ps0 = cps.tile([128, 2, chunk], F32, tag="cps")
                    ps0s.append(ps0)
                    for ty in range(2):
                        y1 = 2 * yga + par + 2 * ty
                        q, t = y1 % 4, y1 // 4
                        nc.tensor.matmul(ps0[:, ty, :], wdr("w1p0"), xtq[q][:, t : t + 2, :],
                                         start=True, stop=True, perf_mode=DR)
                if gi <= 2:  # remainder group g: 2 passes
                    g = gi
                    c1r = psm.tile([128, chunk], F32, tag="sm")
                    nc.tensor.matmul(c1r[:], wdr("w1rA", g=g), xtq[0][:, 2 * g : 2 * g + 2, :],
                                     start=True, stop=False, perf_mode=DR)
                    nc.tensor.matmul(c1r[:], wp[:, _WOFF["w1rB"] + 128 * g : _WOFF["w1rB"] + 128 * g + 128],
                                     xtq[0][:, 2 * g + 2, :], start=False, stop=True)
                if pend is not None and gi == 0:
                    f2p_t = emit_fc2(pend)
                if pend is not None and gi == 2:
                    emit_fc3(a4_t, c - 1)
                    pend = None
                s0, b0 = scb(0, 128)
                nc.scalar.activation(actc[:, 2 * yga : 2 * yga + 3 : 2, :], ps0s[0][:],
                                     AF.Sign, bias=b0, scale=s0)
                nc.vector.tensor_scalar(actc[:, 2 * yga + 1 : 2 * yga + 4 : 2, :],
                                        ps0s[1][:], scp[0:128, 4:5], None, GE)
                if gi <= 2:
                    s1_, b1_ = scb(2, 128)
                    nc.scalar.activation(actc[:, 24 + gi, :], c1r[:], AF.Sign,
                                         bias=b1_, scale=s1_)
                if f2p_t is not None and gi == 1:
                    a4_t = emit_a4(f2p_t)
                    f2p_t = None
                if gi == 2:
                    # 7 phase-shifted bundle copies (16-row shifts); all rem
                    # signs just landed, so the DMAs start ASAP.
                    for p in range(1, 8):
                        ns = 3 if p <= 3 else 2
                        if p <= 3:
                            nc.vector.memset(actc[:, 24 + 3 * p + 2, :], 0.0)
                        nc.sync.dma_start(out=actc[0 : 128 - 16 * p, 24 + 3 * p : 24 + 3 * p + ns, :],
                                          in_=actc[16 * p : 128, 24 : 24 + ns, :])
                        nc.sync.dma_start(out=actc[128 - 16 * p : 128, 24 + 3 * p : 24 + 3 * p + 2, :],
                                          in_=actc[0 : 16 * p, 25 : 27, :])
                    if c + 1 < n_chunks:  # issue next chunk's input loads
                        xtq_next = load_x(c + 1)
                if stage >= 3:
                    for pi in ILV.get(gi, []):
                        c2pair(0, *PAIRS2[pi])
            if stage <= 2:
                dt_ = dpool.tile([128, 512], F32, tag="dbg")
                nc.vector.tensor_copy(out=dt_[:], in_=actc[:, 0, 0:512])
                nc.sync.dma_start(out=dbg[:], in_=dt_[:])
                continue

            # ---- conv2 Mt1 (Mt0 ran interleaved above) ----
            for ya, yb in PAIRS2:
                c2pair(1, ya, yb)
            for p in range(10):  # Mt2 merged: 4 DR passes per pair; scalar act
                ya = 2 * p
                ps = psm.tile([128, chunk], F32, tag="sm", name="m2ps")
                for j in range(3):
                    nc.tensor.matmul(ps[:], wdr("wm2", j=j),
                                     actc[:, ya + 2 * j : ya + 2 * j + 2, :],
                                     start=(j == 0), stop=False, perf_mode=DR)
                sa = 24 + 3 * (ya % 8) + ya // 8
                nc.tensor.matmul(ps[:], wdr("wr2m"), actc[:, sa : sa + 4 : 3, :],
                                 start=False, stop=True, perf_mode=DR)
                s2_, b2_ = scb(9, 128)
                nc.scalar.activation(act2[:, 5 * p + 4, :], ps[:], AF.Sign,
                                     bias=b2_, scale=s2_)
            if stage <= 3:
                dt_ = dpool.tile([128, 512], F32, tag="dbg")
                nc.vector.tensor_copy(out=dt_[:], in_=act2[:, 0, 0:512])
                nc.sync.dma_start(out=dbg[:], in_=dt_[:])
                continue

            # ---- fc1: 25 DR passes over 50 full K-tiles ----
            f1ps = psm.tile([128, chunk], F32, tag="sm", name="f1ps")
            k = 0
            for p in range(10):
                for off in (0, 2):
                    b = 5 * p + off
                    nc.tensor.matmul(f1ps[:], w3t[:, b : b + 2, :], act2[:, b : b + 2, :],
                                     start=(k == 0), stop=False, perf_mode=DR)
                    k += 1
            for q in range(5):
                b = 10 * q + 4
                nc.tensor.matmul(f1ps[:], w3t[:, b : b + 6 : 5, :], act2[:, b : b + 6 : 5, :],
                                 start=False, stop=(q == 4), perf_mode=DR)
            a3 = fpool.tile([120, chunk], BF16, tag="a3")
            s3_, b3_ = scb(11, 120)
            nc.scalar.activation(a3[:], f1ps[0:120, :], AF.Sign, bias=b3_, scale=s3_)
            if stage <= 35:
                dt_ = dpool.tile([128, 512], F32, tag="dbg")
                nc.any.memset(dt_[:], 0.0)
                nc.vector.tensor_copy(out=dt_[0:120, :], in_=a3[:, 0:512])
                nc.sync.dma_start(out=dbg[:], in_=dt_[:])
                continue
            pend = a3  # fc2/fc3 pipelined into the next chunk (or the epilogue)

        if stage >= 37 and pend is not None:
            emit_fc3(emit_a4(emit_fc2(pend)), n_chunks - 1)

    nc.compile()
    return nc


def kernel(**inputs):
    inputs = {k: np.asarray(v) for k, v in inputs.items()}
    consts = build_consts(inputs)
    nc = build_nc(consts)
    xs = prep_x(inputs["x"].astype(np.float32))
    in_maps = [{"xt": xs[i]} for i in range(N_CORES)]
    res = run_bass_kernel_spmd(nc, in_maps, core_ids=list(range(N_CORES)))
    out = np.concatenate([np.asarray(r["out"]).astype(np.float32).T for r in res.results], axis=0)
    return out.astype(np.float32)
